# revision 4
# baseline (speedup 1.0000x reference)
"""Trainium2 Bass kernel for the FHE BSGS conv-as-linear-transform problem.

Computes, for each batch row b of x[64, 65536]:
    out[b, s] = sum_{i=0}^{15} x[b, (s + 2^i * stride) % S] * diagonals[i, s]
    out = roll(out, S // (2 * reps))            (S = 65536)

Distribution: batch dim sharded across 8 NeuronCores (8 rows per core),
diagonals + stationary matrices replicated. No cross-core communication.

Per-core algorithm (free-minor layout: slot s = 512*p + f, p = partition):
  - A shift delta = 512*m + df decomposes into a free-dim column offset df
    (read the x tile at offset df against a 256-column halo) and a partition
    rotation by m (folded into the accumulation matmul).
  - The 16 elementwise products run on TWO engines in parallel: DVE (f16
    2x mode, ~0.52 ns/col) takes the even-offset transforms fused into
    arithmetic-progression groups; the Pool engine (~2 ns/col, but no
    2x-alignment constraint) takes the odd-offset transforms plus enough
    even ones to balance (~3 of 16 for stride=1).  Products are split by
    batch half so compute starts after half the x tile has landed.
  - TensorE accumulates every product tile into a per-batch PSUM bank via a
    128x128 rotation-permutation stationary matmul; the partition rotation
    of each shift AND the final roll (multiple of 512 slots) are folded in,
    so PSUM holds the final rolled output directly.  PE matmuls are emitted
    in estimated product-availability order (engines run in-order).
  - ScalarE (Act) evicts each PSUM bank to f16 as soon as its last matmul
    lands; output DMA per batch-half.  y is stored f16 and upcast on host.

All device-input arrays are prepared host-side in the exact SBUF layout so
every input DMA is a dense partition-major copy.
"""

import os
import sys

for _p in ("/opt/trn_rl_repo",):
    if os.path.isdir(_p) and _p not in sys.path:
        sys.path.insert(0, _p)

import numpy as np

import concourse.bass as bass
import concourse.mybir as mybir
from concourse import bacc
from concourse.bass_utils import run_bass_kernel_spmd
from concourse.tile import TileContext

N_CORES = 8
BATCH = 64
SLOTS = 65536
NUM_T = 16
P = 128                 # SBUF partitions
F = SLOTS // P          # 512 slots per partition per batch
BPC = BATCH // N_CORES  # 8 batch rows per core
HALO = 256              # halo columns (covers df <= 256 for stride=1)
XPITCH = F + HALO       # 768
MAX_GROUP = 6           # max transforms fused into one DVE op
NS_COL_DVE = 0.521      # DVE f16 2x ns/col (cost model)
NS_COL_POOL = 1.984     # Pool f16 ns/col (cost model, 0.42 efficiency)


def _decomp(stride, reps):
    """Per-transform (m_i, df_i) shift decomposition + stationary rotations."""
    roll = (SLOTS // (2 * reps)) % SLOTS
    assert roll % F == 0, f"final roll {roll} not a multiple of {F}"
    mr = roll // F
    dec = []
    for i in range(NUM_T):
        delta = ((1 << i) * stride) % SLOTS
        dec.append((delta // F, delta % F))
    rots = [(m - mr) % P for (m, _) in dec]
    uniq = sorted(set(rots))
    sidx = {a: j for j, a in enumerate(uniq)}
    return dec, rots, uniq, sidx


def _plan(stride, reps):
    """Split transforms between DVE and Pool; group DVE transforms.

    Returns (dec, rots, uniq, sidx, pool_idx, dve_groups) where dve_groups
    is a list of (base, step, members) fused ops (offsets base + k*step,
    base and step even) and pool_idx are the Pool-engine transforms.
    """
    dec, rots, uniq, sidx = _decomp(stride, reps)
    odd = [i for i in range(NUM_T) if dec[i][1] % 2 == 1]
    pool = list(odd)
    # balance-pad Pool with df==0 transforms (largest index first)
    k_opt = max(len(pool), int(round(NUM_T * NS_COL_DVE /
                                     (NS_COL_DVE + NS_COL_POOL))))
    zeros = [i for i in range(NUM_T) if dec[i][1] == 0 and i not in pool]
    while len(pool) < k_opt and zeros:
        pool.append(zeros.pop())
    rest = [i for i in range(NUM_T) if i not in pool]
    # chain equal-step even progressions over sorted offsets
    rows = sorted(rest, key=lambda i: dec[i][1])
    groups = []
    k = 0
    while k < len(rows):
        base = dec[rows[k]][1]
        members = [rows[k]]
        step = None
        j = k + 1
        while j < len(rows) and len(members) < MAX_GROUP:
            d = dec[rows[j]][1] - dec[rows[j - 1]][1]
            if d % 2 != 0:
                break
            if step is None:
                step = d
            elif d != step:
                break
            members.append(rows[j])
            j += 1
        groups.append((base, step or 0, members))
        k = j
    # execution order: smallest group first (fast PE start), then largest,
    # then the rest ascending so the tail op is small.
    groups.sort(key=lambda g: len(g[2]))
    if len(groups) > 1:
        first = groups[0]
        biggest = groups[-1]
        mid = groups[1:-1]
        groups = [first, biggest] + mid
    return dec, rots, uniq, sidx, pool, groups


def build_module(stride=1, reps=1, debug=False, repeat=1):
    """Build + finalize the per-core Bass module (same program on all cores)."""
    dec, rots, uniq, sidx, pool_idx, groups = _plan(stride, reps)
    ns = len(uniq)
    HB = BPC // 2  # batches per half

    # ds column layout: [G1 diags | pool0 diag | stats | GA.. | rest | pools]
    # chunk1 = first group's diags + first pool diag + stats (feeds the
    # start); chunk2 = biggest group's diags; chunk3 = everything else.
    dd_order = list(groups[0][2]) + pool_idx[:1]
    c1_blocks = len(dd_order)
    for g in groups[1:]:
        dd_order += g[2]
    c2_blocks = len(groups[1][2]) if len(groups) > 1 else 0
    dd_order += pool_idx[1:]
    dd_col = {i: k for k, i in enumerate(dd_order)}
    st0 = c1_blocks * F                    # stats start (after chunk1 diags)
    dcol = lambda i: (dd_col[i] * F if dd_col[i] < c1_blocks
                      else dd_col[i] * F + ns * P)
    ds_cols = NUM_T * F + ns * P
    n_c1 = st0 + ns * P
    n_c2 = n_c1 + c2_blocks * F

    f16 = mybir.dt.float16
    f32 = mybir.dt.float32

    nc = bacc.Bacc("TRN2", target_bir_lowering=False, debug=debug,
                   num_devices=N_CORES)
    xh = nc.dram_tensor("xh", [P, BPC * XPITCH], f16, kind="ExternalInput")
    ds = nc.dram_tensor("ds", [P, ds_cols], f16, kind="ExternalInput")
    y = nc.dram_tensor("y", [BPC, SLOTS], f16, kind="ExternalOutput")

    with TileContext(nc) as tc:
        with (
            tc.tile_pool(name="xa", bufs=1) as xa_pool,
            tc.tile_pool(name="dda", bufs=1) as dd_pool,
            tc.tile_pool(name="prod", bufs=3) as prod_pool,
            tc.tile_pool(name="pprod", bufs=2) as pprod_pool,
            tc.tile_pool(name="outs", bufs=2) as out_pool,
            tc.tile_pool(name="ps", bufs=1, space="PSUM") as ps_pool,
        ):
            DS = dd_pool.tile([P, ds_cols], f16)
            X = xa_pool.tile([P, BPC * XPITCH], f16)
            xmid = HB * XPITCH
            # Act: chunk1 diags+stats, then X half2.  SP: X half1, then
            # chunk2 (biggest group's diags), then the rest.
            nc.scalar.dma_start(out=DS[:, :n_c1], in_=ds[:, :n_c1])
            nc.sync.dma_start(out=X[:, :xmid], in_=xh[:, :xmid])
            nc.scalar.dma_start(out=X[:, xmid:], in_=xh[:, xmid:])
            nc.sync.dma_start(out=DS[:, n_c1:n_c2], in_=ds[:, n_c1:n_c2])
            if n_c2 < ds_cols:
                nc.sync.dma_start(out=DS[:, n_c2:], in_=ds[:, n_c2:])

            max_prod = max(len(g[2]) for g in groups) * HB * F

            def body(_iv=None):
                psums = [
                    ps_pool.tile([P, F], f32, name=f"psum{b}", tag=f"ps{b}",
                                 bufs=1)
                    for b in range(BPC)
                ]
                # ---- product op list: (eng, half, members, base, step) ----
                dve_ops = []
                for h in (0, 1):
                    gl = list(groups)
                    if h == 1 and len(gl[-1][2]) > 1:
                        # split the final op into singletons for a short tail
                        last = gl.pop()
                        gl += [(last[0] + k * last[1], 0, [i])
                               for k, i in enumerate(last[2])]
                    for base, step, mem in gl:
                        dve_ops.append((h, base, step, mem))
                pool_ops = []
                for h in (0, 1):
                    for i in pool_idx:
                        pool_ops.append((h, dec[i][1], 0, [i]))

                # estimated ready times (ns) for PE emission ordering
                ready = {}
                t = 2500.0
                for h, base, step, mem in dve_ops:
                    t += len(mem) * HB * F * NS_COL_DVE + 180
                    for k, i in enumerate(mem):
                        ready[(i, h)] = t
                t = 2500.0
                for h, base, step, mem in pool_ops:
                    t += len(mem) * HB * F * NS_COL_POOL + 150
                    ready[(mem[0], h)] = t

                # ---- emit product ops; record tiles ----
                ptile = {}   # (i, h) -> (tile, col0)
                for ops, pool_, eng in ((dve_ops, prod_pool, nc.vector),
                                        (pool_ops, pprod_pool, nc.gpsimd)):
                    for h, base, step, mem in ops:
                        ng = len(mem)
                        prod = pool_.tile(
                            [P, ng * HB * F], f16, name="prod",
                            tag=f"prod{eng.engine.value}",
                            padded_shape=[P, max_prod])
                        in0 = bass.AP(
                            X.tensor,
                            X.offset + base + h * HB * XPITCH,
                            [list(X.ap[0]), [step, ng], [XPITCH, HB], [1, F]],
                        )
                        c0 = dcol(mem[0])
                        in1 = bass.AP(
                            DS.tensor, DS.offset + c0,
                            [list(DS.ap[0]), [F, ng], [0, HB], [1, F]],
                        )
                        out4 = prod[:].rearrange("p (g b f) -> p g b f",
                                                 b=HB, f=F)
                        eng.tensor_mul(out4, in0, in1)
                        for k, i in enumerate(mem):
                            ptile[(i, h)] = (prod, k * HB * F)

                # ---- PE matmuls in estimated-availability order ----
                order = sorted(ptile.keys(), key=lambda ih: ready[ih])
                seen = set()
                last = {}
                for b in range(BPC):
                    h = b // HB
                    last[b] = [ih for ih in order if ih[1] == h][-1]
                for i, h in order:
                    prod, c0 = ptile[(i, h)]
                    lhsT = DS[:, st0 + sidx[rots[i]] * P:
                              st0 + (sidx[rots[i]] + 1) * P]
                    for k in range(HB):
                        b = h * HB + k
                        nc.tensor.matmul(
                            psums[b][:], lhsT,
                            prod[:, c0 + k * F:c0 + (k + 1) * F],
                            start=(b not in seen),
                            stop=((i, h) == last[b]),
                        )
                        seen.add(b)

                # ---- eviction (Act) + output DMA per batch half ----
                for h in range(2):
                    ot = out_pool.tile([P, HB * F], f16, name=f"ot{h}",
                                       tag=f"ot{h}")
                    for k in range(HB):
                        b = h * HB + k
                        nc.scalar.copy(ot[:, k * F:(k + 1) * F], psums[b][:])
                    ydst = y[h * HB:(h + 1) * HB, :].rearrange(
                        "b (p f) -> p b f", f=F)
                    eng = nc.sync if h == 0 else nc.scalar
                    eng.dma_start(out=ydst, in_=ot[:].rearrange(
                        "p (b f) -> p b f", f=F))

            if repeat == 1:
                body()
            else:
                with tc.For_i(0, repeat, 1):
                    body()
    nc.finalize()
    return nc


def prep_inputs(x, diagonals, stride=1, reps=1):
    """Host-side shard + relayout. Returns in_maps for run_bass_kernel_spmd."""
    dec, rots, uniq, sidx, pool_idx, groups = _plan(stride, reps)
    ns = len(uniq)

    x16 = np.ascontiguousarray(x, dtype=np.float16)
    # halo tiles in SBUF layout: xh[p, b*XPITCH + j] = x[b, (512p + j) % S]
    j = np.arange(XPITCH)
    idx = (np.arange(P)[:, None] * F + j[None, :]) % SLOTS
    xt = x16[:, idx]                       # [BATCH, P, XPITCH]
    xh = np.ascontiguousarray(
        np.stack([np.transpose(xt[c * BPC:(c + 1) * BPC], (1, 0, 2))
                  .reshape(P, BPC * XPITCH) for c in range(N_CORES)])
    )

    # ds: [chunk1 diag blocks | stats | remaining diag blocks], with each
    # diag block pre-rotated along partitions by its shift's m.
    d16 = np.asarray(diagonals, dtype=np.float16).reshape(NUM_T, P, F)
    dd_order = list(groups[0][2]) + pool_idx[:1]
    for g in groups[1:]:
        dd_order += g[2]
    dd_order += pool_idx[1:]
    c1_blocks = len(groups[0][2]) + 1
    ddl = [np.roll(d16[i], dec[i][0], axis=0) for i in dd_order]
    dd = np.transpose(np.stack(ddl), (1, 0, 2)).reshape(P, NUM_T * F)

    st = np.zeros((ns, P, P), np.float16)
    cols = np.arange(P)
    for k, a in enumerate(uniq):
        st[k, (cols + a) % P, cols] = 1.0
    st = np.transpose(st, (1, 0, 2)).reshape(P, ns * P)

    c1 = c1_blocks * F
    ds = np.ascontiguousarray(
        np.concatenate([dd[:, :c1], st, dd[:, c1:]], axis=1))

    in_maps = []
    for c in range(N_CORES):
        in_maps.append({"xh": xh[c], "ds": ds})
    return in_maps


_MODULE_CACHE = {}


def kernel(**inputs):
    x = np.asarray(inputs["x"], dtype=np.float32)
    diagonals = np.asarray(inputs["diagonals"], dtype=np.float32)
    stride = int(np.asarray(inputs.get("stride", 1)))
    reps = int(np.asarray(inputs.get("reps", 1)))
    assert x.shape == (BATCH, SLOTS) and diagonals.shape == (NUM_T, SLOTS)
    # halo must cover the largest in-partition shift
    dec, _, _, _ = _decomp(stride, reps)
    assert max(df for _, df in dec) <= HALO, "halo too small for this stride"

    key = (stride, reps)
    if key not in _MODULE_CACHE:
        _MODULE_CACHE[key] = build_module(stride, reps)
    nc = _MODULE_CACHE[key]

    in_maps = prep_inputs(x, diagonals, stride, reps)
    res = run_bass_kernel_spmd(nc, in_maps, list(range(N_CORES)))
    out = np.concatenate(
        [np.asarray(res.results[c]["y"]) for c in range(N_CORES)], axis=0
    )
    return out.astype(np.float32)


# revision 7
# speedup vs baseline: 1.4987x; 1.4987x over previous
"""Trainium2 Bass kernel for the FHE BSGS conv-as-linear-transform problem.

Computes, for each batch row b of x[64, 65536]:
    out[b, s] = sum_{i=0}^{15} x[b, (s + 2^i * stride) % S] * diagonals[i, s]
    out = roll(out, S // (2 * reps))            (S = 65536)

Distribution: batch dim sharded across 8 NeuronCores (8 rows per core),
diagonals + stationary matrices replicated. No cross-core communication.

Per-core algorithm (free-minor layout: slot s = 512*p + f, p = partition):
  - A shift delta = 512*m + df decomposes into a free-dim column offset df
    (read the x tile at offset df against a 256-column halo) and a partition
    rotation by m (folded into the accumulation matmul).
  - The 16 elementwise products run on TWO engines in parallel: DVE (f16
    2x mode, ~0.52 ns/col) takes the even-offset transforms fused into
    arithmetic-progression groups; the Pool engine (~2 ns/col, but no
    2x-alignment constraint) takes the odd-offset transforms plus enough
    even ones to balance (~3 of 16 for stride=1).  Products are split by
    batch half so compute starts after half the x tile has landed.
  - TensorE accumulates every product tile into a per-batch PSUM bank via a
    128x128 rotation-permutation stationary matmul; the partition rotation
    of each shift AND the final roll (multiple of 512 slots) are folded in,
    so PSUM holds the final rolled output directly.  PE matmuls are emitted
    in estimated product-availability order (engines run in-order).
  - ScalarE (Act) evicts each PSUM bank to f16 as soon as its last matmul
    lands; output DMA per batch-half.  y is stored f16 and upcast on host.

All device-input arrays are prepared host-side in the exact SBUF layout so
every input DMA is a dense partition-major copy.
"""

import os
import sys

for _p in ("/opt/trn_rl_repo",):
    if os.path.isdir(_p) and _p not in sys.path:
        sys.path.insert(0, _p)

import numpy as np

import concourse.bass as bass
import concourse.mybir as mybir
from concourse import bacc
from concourse.bass_utils import run_bass_kernel_spmd
from concourse.tile import TileContext

N_CORES = 8
BATCH = 64
SLOTS = 65536
NUM_T = 16
P = 128                 # SBUF partitions
F = SLOTS // P          # 512 slots per partition per batch
BPC = BATCH // N_CORES  # 8 batch rows per core
HALO = 256              # halo columns (covers df <= 256 for stride=1)
XPITCH = F + HALO       # 768
MAX_GROUP = 6           # max transforms fused into one DVE op
NS_COL_DVE = 0.521      # DVE f16 2x ns/col (cost model)
NS_COL_POOL = 1.984     # Pool f16 ns/col (cost model, 0.42 efficiency)


def _decomp(stride, reps):
    """Per-transform (m_i, df_i) shift decomposition + stationary rotations."""
    roll = (SLOTS // (2 * reps)) % SLOTS
    assert roll % F == 0, f"final roll {roll} not a multiple of {F}"
    mr = roll // F
    dec = []
    for i in range(NUM_T):
        delta = ((1 << i) * stride) % SLOTS
        dec.append((delta // F, delta % F))
    rots = [(m - mr) % P for (m, _) in dec]
    uniq = sorted(set(rots))
    sidx = {a: j for j, a in enumerate(uniq)}
    return dec, rots, uniq, sidx


KNOB_POOL = int(os.environ.get("K_POOL", "3"))    # transforms on Pool engine


def _plan(stride, reps):
    """Split transforms between DVE and Pool; group DVE transforms.

    Returns (dec, rots, uniq, sidx, pool_idx, dve_groups) where dve_groups
    is a list of (base, step, members) fused ops (offsets base + k*step,
    base and step even) and pool_idx are the Pool-engine transforms.
    """
    dec, rots, uniq, sidx = _decomp(stride, reps)
    odd = [i for i in range(NUM_T) if dec[i][1] % 2 == 1]
    pool = list(odd) if KNOB_POOL > 0 else []
    # balance-pad Pool with df==0 transforms (largest index first)
    k_opt = max(len(pool), KNOB_POOL)
    zeros = [i for i in range(NUM_T) if dec[i][1] == 0 and i not in pool]
    while len(pool) < k_opt and zeros:
        pool.append(zeros.pop())
    rest = [i for i in range(NUM_T) if i not in pool]
    # chain equal-step even progressions over sorted offsets
    rows = sorted(rest, key=lambda i: dec[i][1])
    groups = []
    k = 0
    while k < len(rows):
        base = dec[rows[k]][1]
        members = [rows[k]]
        step = None
        j = k + 1
        while j < len(rows) and len(members) < MAX_GROUP:
            d = dec[rows[j]][1] - dec[rows[j - 1]][1]
            if d % 2 != 0:
                break
            if step is None:
                step = d
            elif d != step:
                break
            members.append(rows[j])
            j += 1
        groups.append((base, step or 0, members))
        k = j
    # execution order: smallest group first (fast PE start), then largest,
    # then the rest ascending so the tail op is small.
    groups.sort(key=lambda g: len(g[2]))
    if len(groups) > 1:
        first = groups[0]
        biggest = groups[-1]
        mid = groups[1:-1]
        groups = [first, biggest] + mid
    return dec, rots, uniq, sidx, pool, groups


def build_module(stride=1, reps=1, debug=False, repeat=1):
    """Build + finalize the per-core Bass module (same program on all cores)."""
    dec, rots, uniq, sidx, pool_idx, groups = _plan(stride, reps)
    ns = len(uniq)
    HB = BPC // 2  # batches per half

    # ds column layout: [G1 diags | pool0 diag | stats | GA.. | rest | pools]
    # chunk1 = first group's diags + first pool diag + stats (feeds the
    # start); chunk2 = biggest group's diags; chunk3 = everything else.
    dd_order = list(groups[0][2]) + pool_idx[:1]
    c1_blocks = len(dd_order)
    for g in groups[1:]:
        dd_order += g[2]
    c2_blocks = len(groups[1][2]) if len(groups) > 1 else 0
    dd_order += pool_idx[1:]
    dd_col = {i: k for k, i in enumerate(dd_order)}
    st0 = c1_blocks * F                    # stats start (after chunk1 diags)
    dcol = lambda i: (dd_col[i] * F if dd_col[i] < c1_blocks
                      else dd_col[i] * F + ns * P)
    ds_cols = NUM_T * F + ns * P
    n_c1 = st0 + ns * P
    n_c2 = n_c1 + c2_blocks * F

    f16 = mybir.dt.float16
    f32 = mybir.dt.float32

    nc = bacc.Bacc("TRN2", target_bir_lowering=False, debug=debug,
                   num_devices=N_CORES)
    xh = nc.dram_tensor("xh", [P, BPC * XPITCH], f16, kind="ExternalInput")
    ds = nc.dram_tensor("ds", [P, ds_cols], f16, kind="ExternalInput")
    y = nc.dram_tensor("y", [BPC, SLOTS], f16, kind="ExternalOutput")

    with TileContext(nc) as tc:
        with (
            tc.tile_pool(name="xa", bufs=1) as xa_pool,
            tc.tile_pool(name="dda", bufs=1) as dd_pool,
            tc.tile_pool(name="prod", bufs=3) as prod_pool,
            tc.tile_pool(name="pprod", bufs=2) as pprod_pool,
            tc.tile_pool(name="outs", bufs=2) as out_pool,
            tc.tile_pool(name="ps", bufs=1, space="PSUM") as ps_pool,
        ):
            DS = dd_pool.tile([P, ds_cols], f16)
            X = xa_pool.tile([P, BPC * XPITCH], f16)
            xmid = HB * XPITCH
            # Act: chunk1 diags+stats, then X half2.  SP: X half1, then
            # chunk2 (biggest group's diags), then the rest.
            nc.scalar.dma_start(out=DS[:, :n_c1], in_=ds[:, :n_c1])
            nc.sync.dma_start(out=X[:, :xmid], in_=xh[:, :xmid])
            nc.scalar.dma_start(out=X[:, xmid:], in_=xh[:, xmid:])
            nc.sync.dma_start(out=DS[:, n_c1:n_c2], in_=ds[:, n_c1:n_c2])
            if n_c2 < ds_cols:
                nc.sync.dma_start(out=DS[:, n_c2:], in_=ds[:, n_c2:])

            max_prod = max(len(g[2]) for g in groups) * HB * F

            def body(_iv=None):
                psums = [
                    ps_pool.tile([P, F], f32, name=f"psum{b}", tag=f"ps{b}",
                                 bufs=1)
                    for b in range(BPC)
                ]
                # ---- product op list: (eng, half, members, base, step) ----
                dve_ops = []
                for h in (0, 1):
                    gl = list(groups)
                    if h == 1 and len(gl[-1][2]) > 1:
                        # split the final op into singletons for a short tail
                        last = gl.pop()
                        gl += [(last[0] + k * last[1], 0, [i])
                               for k, i in enumerate(last[2])]
                    for base, step, mem in gl:
                        dve_ops.append((h, base, step, mem))
                pool_ops = []
                for h in (0, 1):
                    for i in pool_idx:
                        pool_ops.append((h, dec[i][1], 0, [i]))

                # estimated ready times (ns) for PE emission ordering
                ready = {}
                t = 2500.0
                for h, base, step, mem in dve_ops:
                    t += len(mem) * HB * F * NS_COL_DVE + 180
                    for k, i in enumerate(mem):
                        ready[(i, h)] = t
                t = 2500.0
                for h, base, step, mem in pool_ops:
                    t += len(mem) * HB * F * NS_COL_POOL + 150
                    ready[(mem[0], h)] = t

                # ---- emit product ops; record tiles ----
                ptile = {}   # (i, h) -> (tile, col0)
                for ops, pool_, eng in ((dve_ops, prod_pool, nc.vector),
                                        (pool_ops, pprod_pool, nc.gpsimd)):
                    for h, base, step, mem in ops:
                        ng = len(mem)
                        prod = pool_.tile(
                            [P, ng * HB * F], f16, name="prod",
                            tag=f"prod{eng.engine.value}",
                            padded_shape=[P, max_prod])
                        in0 = bass.AP(
                            X.tensor,
                            X.offset + base + h * HB * XPITCH,
                            [list(X.ap[0]), [step, ng], [XPITCH, HB], [1, F]],
                        )
                        c0 = dcol(mem[0])
                        in1 = bass.AP(
                            DS.tensor, DS.offset + c0,
                            [list(DS.ap[0]), [F, ng], [0, HB], [1, F]],
                        )
                        out4 = prod[:].rearrange("p (g b f) -> p g b f",
                                                 b=HB, f=F)
                        eng.tensor_mul(out4, in0, in1)
                        for k, i in enumerate(mem):
                            ptile[(i, h)] = (prod, k * HB * F)

                # ---- PE matmuls in estimated-availability order ----
                order = sorted(ptile.keys(), key=lambda ih: ready[ih])
                seen = set()
                last = {}
                for b in range(BPC):
                    h = b // HB
                    last[b] = [ih for ih in order if ih[1] == h][-1]
                for i, h in order:
                    prod, c0 = ptile[(i, h)]
                    lhsT = DS[:, st0 + sidx[rots[i]] * P:
                              st0 + (sidx[rots[i]] + 1) * P]
                    for k in range(HB):
                        b = h * HB + k
                        nc.tensor.matmul(
                            psums[b][:], lhsT,
                            prod[:, c0 + k * F:c0 + (k + 1) * F],
                            start=(b not in seen),
                            stop=((i, h) == last[b]),
                        )
                        seen.add(b)

                # ---- eviction (Act) + output DMA per batch half ----
                for h in range(2):
                    ot = out_pool.tile([P, HB * F], f16, name=f"ot{h}",
                                       tag=f"ot{h}")
                    for k in range(HB):
                        b = h * HB + k
                        nc.scalar.copy(ot[:, k * F:(k + 1) * F], psums[b][:])
                    ydst = y[h * HB:(h + 1) * HB, :].rearrange(
                        "b (p f) -> p b f", f=F)
                    eng = nc.sync if h == 0 else nc.scalar
                    eng.dma_start(out=ydst, in_=ot[:].rearrange(
                        "p (b f) -> p b f", f=F))

            if repeat == 1:
                body()
            else:
                with tc.For_i(0, repeat, 1):
                    body()
    nc.finalize()
    return nc


def prep_inputs(x, diagonals, stride=1, reps=1):
    """Host-side shard + relayout. Returns in_maps for run_bass_kernel_spmd."""
    dec, rots, uniq, sidx, pool_idx, groups = _plan(stride, reps)
    ns = len(uniq)

    x16 = np.ascontiguousarray(x, dtype=np.float16)
    # halo tiles in SBUF layout: xh[p, b*XPITCH + j] = x[b, (512p + j) % S]
    j = np.arange(XPITCH)
    idx = (np.arange(P)[:, None] * F + j[None, :]) % SLOTS
    xt = x16[:, idx]                       # [BATCH, P, XPITCH]
    xh = np.ascontiguousarray(
        np.stack([np.transpose(xt[c * BPC:(c + 1) * BPC], (1, 0, 2))
                  .reshape(P, BPC * XPITCH) for c in range(N_CORES)])
    )

    # ds: [chunk1 diag blocks | stats | remaining diag blocks], with each
    # diag block pre-rotated along partitions by its shift's m.
    d16 = np.asarray(diagonals, dtype=np.float16).reshape(NUM_T, P, F)
    dd_order = list(groups[0][2]) + pool_idx[:1]
    for g in groups[1:]:
        dd_order += g[2]
    dd_order += pool_idx[1:]
    c1_blocks = len(groups[0][2]) + len(pool_idx[:1])
    ddl = [np.roll(d16[i], dec[i][0], axis=0) for i in dd_order]
    dd = np.transpose(np.stack(ddl), (1, 0, 2)).reshape(P, NUM_T * F)

    st = np.zeros((ns, P, P), np.float16)
    cols = np.arange(P)
    for k, a in enumerate(uniq):
        st[k, (cols + a) % P, cols] = 1.0
    st = np.transpose(st, (1, 0, 2)).reshape(P, ns * P)

    c1 = c1_blocks * F
    ds = np.ascontiguousarray(
        np.concatenate([dd[:, :c1], st, dd[:, c1:]], axis=1))

    in_maps = []
    for c in range(N_CORES):
        in_maps.append({"xh": xh[c], "ds": ds})
    return in_maps


_MODULE_CACHE = {}


def kernel(**inputs):
    x = np.asarray(inputs["x"], dtype=np.float32)
    diagonals = np.asarray(inputs["diagonals"], dtype=np.float32)
    stride = int(np.asarray(inputs.get("stride", 1)))
    reps = int(np.asarray(inputs.get("reps", 1)))
    assert x.shape == (BATCH, SLOTS) and diagonals.shape == (NUM_T, SLOTS)
    # halo must cover the largest in-partition shift
    dec, _, _, _ = _decomp(stride, reps)
    assert max(df for _, df in dec) <= HALO, "halo too small for this stride"

    key = (stride, reps)
    if key not in _MODULE_CACHE:
        _MODULE_CACHE[key] = build_module(stride, reps)
    nc = _MODULE_CACHE[key]

    in_maps = prep_inputs(x, diagonals, stride, reps)
    res = run_bass_kernel_spmd(nc, in_maps, list(range(N_CORES)))
    out = np.concatenate(
        [np.asarray(res.results[c]["y"]) for c in range(N_CORES)], axis=0
    )
    return out.astype(np.float32)


# revision 9
# speedup vs baseline: 1.5129x; 1.0095x over previous
"""Trainium2 Bass kernel for the FHE BSGS conv-as-linear-transform problem.

Computes, for each batch row b of x[64, 65536]:
    out[b, s] = sum_{i=0}^{15} x[b, (s + 2^i * stride) % S] * diagonals[i, s]
    out = roll(out, S // (2 * reps))            (S = 65536)

Distribution: batch dim sharded across 8 NeuronCores (8 rows per core),
diagonals + stationary matrices replicated. No cross-core communication.

Per-core algorithm (free-minor layout: slot s = 512*p + f, p = partition):
  - A shift delta = 512*m + df decomposes into a free-dim column offset df
    (read the x tile at offset df against a 256-column halo) and a partition
    rotation by m (folded into the accumulation matmul).
  - The 16 elementwise products run on TWO engines in parallel: DVE (f16
    2x mode, ~0.52 ns/col) takes the even-offset transforms fused into
    arithmetic-progression groups; the Pool engine (~2 ns/col, but no
    2x-alignment constraint) takes the odd-offset transforms plus enough
    even ones to balance (~3 of 16 for stride=1).  Products are split by
    batch half so compute starts after half the x tile has landed.
  - TensorE accumulates every product tile into a per-batch PSUM bank via a
    128x128 rotation-permutation stationary matmul; the partition rotation
    of each shift AND the final roll (multiple of 512 slots) are folded in,
    so PSUM holds the final rolled output directly.  PE matmuls are emitted
    in estimated product-availability order (engines run in-order).
  - ScalarE (Act) evicts each PSUM bank to f16 as soon as its last matmul
    lands; output DMA per batch-half.  y is stored f16 and upcast on host.

All device-input arrays are prepared host-side in the exact SBUF layout so
every input DMA is a dense partition-major copy.
"""

import os
import sys

for _p in ("/opt/trn_rl_repo",):
    if os.path.isdir(_p) and _p not in sys.path:
        sys.path.insert(0, _p)

import numpy as np

import concourse.bass as bass
import concourse.mybir as mybir
from concourse import bacc
from concourse.bass_utils import run_bass_kernel_spmd
from concourse.tile import TileContext

N_CORES = 8
BATCH = 64
SLOTS = 65536
NUM_T = 16
P = 128                 # SBUF partitions
F = SLOTS // P          # 512 slots per partition per batch
BPC = BATCH // N_CORES  # 8 batch rows per core
HALO = 256              # halo columns (covers df <= 256 for stride=1)
XPITCH = F + HALO       # 768
MAX_GROUP = 6           # max transforms fused into one DVE op
NS_COL_DVE = 0.521      # DVE f16 2x ns/col (cost model)
NS_COL_POOL = 1.984     # Pool f16 ns/col (cost model, 0.42 efficiency)


def _decomp(stride, reps):
    """Per-transform (m_i, df_i) shift decomposition + stationary rotations."""
    roll = (SLOTS // (2 * reps)) % SLOTS
    assert roll % F == 0, f"final roll {roll} not a multiple of {F}"
    mr = roll // F
    dec = []
    for i in range(NUM_T):
        delta = ((1 << i) * stride) % SLOTS
        dec.append((delta // F, delta % F))
    rots = [(m - mr) % P for (m, _) in dec]
    uniq = sorted(set(rots))
    sidx = {a: j for j, a in enumerate(uniq)}
    return dec, rots, uniq, sidx


KNOB_POOL = int(os.environ.get("K_POOL", "0"))    # transforms on Pool engine
KNOB_SWAP = int(os.environ.get("K_SWAP", "1"))    # diag as in0, X as in1


def _plan(stride, reps):
    """Split transforms between DVE and Pool; group DVE transforms.

    Returns (dec, rots, uniq, sidx, pool_idx, dve_groups) where dve_groups
    is a list of (base, step, members) fused ops (offsets base + k*step,
    base and step even) and pool_idx are the Pool-engine transforms.
    """
    dec, rots, uniq, sidx = _decomp(stride, reps)
    odd = [i for i in range(NUM_T) if dec[i][1] % 2 == 1]
    pool = list(odd) if KNOB_POOL > 0 else []
    # balance-pad Pool with df==0 transforms (largest index first)
    k_opt = max(len(pool), KNOB_POOL)
    zeros = [i for i in range(NUM_T) if dec[i][1] == 0 and i not in pool]
    while len(pool) < k_opt and zeros:
        pool.append(zeros.pop())
    rest = [i for i in range(NUM_T) if i not in pool]
    # chain equal-step even progressions over sorted offsets
    rows = sorted(rest, key=lambda i: dec[i][1])
    groups = []
    k = 0
    while k < len(rows):
        base = dec[rows[k]][1]
        members = [rows[k]]
        step = None
        j = k + 1
        while j < len(rows) and len(members) < MAX_GROUP:
            d = dec[rows[j]][1] - dec[rows[j - 1]][1]
            if d % 2 != 0:
                break
            if step is None:
                step = d
            elif d != step:
                break
            members.append(rows[j])
            j += 1
        groups.append((base, step or 0, members))
        k = j
    # execution order: smallest group first (fast PE start), then largest,
    # then the rest ascending so the tail op is small.
    groups.sort(key=lambda g: len(g[2]))
    if len(groups) > 1:
        first = groups[0]
        biggest = groups[-1]
        mid = groups[1:-1]
        groups = [first, biggest] + mid
    return dec, rots, uniq, sidx, pool, groups


def build_module(stride=1, reps=1, debug=False, repeat=1):
    """Build + finalize the per-core Bass module (same program on all cores)."""
    dec, rots, uniq, sidx, pool_idx, groups = _plan(stride, reps)
    ns = len(uniq)
    HB = BPC // 2  # batches per half

    # ds column layout: [G1 diags | pool0 diag | stats | GA.. | rest | pools]
    # chunk1 = first group's diags + first pool diag + stats (feeds the
    # start); chunk2 = biggest group's diags; chunk3 = everything else.
    dd_order = list(groups[0][2]) + pool_idx[:1]
    c1_blocks = len(dd_order)
    for g in groups[1:]:
        dd_order += g[2]
    c2_blocks = len(groups[1][2]) if len(groups) > 1 else 0
    dd_order += pool_idx[1:]
    dd_col = {i: k for k, i in enumerate(dd_order)}
    st0 = c1_blocks * F                    # stats start (after chunk1 diags)
    dcol = lambda i: (dd_col[i] * F if dd_col[i] < c1_blocks
                      else dd_col[i] * F + ns * P)
    ds_cols = NUM_T * F + ns * P
    n_c1 = st0 + ns * P
    n_c2 = n_c1 + c2_blocks * F

    f16 = mybir.dt.float16
    f32 = mybir.dt.float32

    nc = bacc.Bacc("TRN2", target_bir_lowering=False, debug=debug,
                   num_devices=N_CORES)
    xh = nc.dram_tensor("xh", [P, BPC * XPITCH], f16, kind="ExternalInput")
    ds = nc.dram_tensor("ds", [P, ds_cols], f16, kind="ExternalInput")
    y = nc.dram_tensor("y", [BPC, SLOTS], f16, kind="ExternalOutput")

    with TileContext(nc) as tc:
        with (
            tc.tile_pool(name="xa", bufs=1) as xa_pool,
            tc.tile_pool(name="dda", bufs=1) as dd_pool,
            tc.tile_pool(name="prod", bufs=3) as prod_pool,
            tc.tile_pool(name="pprod", bufs=2) as pprod_pool,
            tc.tile_pool(name="outs", bufs=2) as out_pool,
            tc.tile_pool(name="ps", bufs=1, space="PSUM") as ps_pool,
        ):
            DS = dd_pool.tile([P, ds_cols], f16)
            X = xa_pool.tile([P, BPC * XPITCH], f16)
            xmid = HB * XPITCH
            # Act: chunk1 diags+stats, then X half2.  SP: X half1, then
            # chunk2 (biggest group's diags), then the rest.
            nc.scalar.dma_start(out=DS[:, :n_c1], in_=ds[:, :n_c1])
            nc.sync.dma_start(out=X[:, :xmid], in_=xh[:, :xmid])
            nc.scalar.dma_start(out=X[:, xmid:], in_=xh[:, xmid:])
            nc.sync.dma_start(out=DS[:, n_c1:n_c2], in_=ds[:, n_c1:n_c2])
            if n_c2 < ds_cols:
                nc.sync.dma_start(out=DS[:, n_c2:], in_=ds[:, n_c2:])

            max_prod = max(len(g[2]) for g in groups) * HB * F

            def body(_iv=None):
                psums = [
                    ps_pool.tile([P, F], f32, name=f"psum{b}", tag=f"ps{b}",
                                 bufs=1)
                    for b in range(BPC)
                ]
                # ---- product op list: (eng, half, members, base, step) ----
                dve_ops = []
                for h in (0, 1):
                    gl = list(groups)
                    if h == 1 and len(gl[-1][2]) > 1:
                        # split the final op into singletons for a short tail
                        last = gl.pop()
                        gl += [(last[0] + k * last[1], 0, [i])
                               for k, i in enumerate(last[2])]
                    for base, step, mem in gl:
                        dve_ops.append((h, base, step, mem))
                pool_ops = []
                for h in (0, 1):
                    for i in pool_idx:
                        pool_ops.append((h, dec[i][1], 0, [i]))

                # estimated ready times (ns) for PE emission ordering
                ready = {}
                t = 2500.0
                for h, base, step, mem in dve_ops:
                    t += len(mem) * HB * F * NS_COL_DVE + 180
                    for k, i in enumerate(mem):
                        ready[(i, h)] = t
                t = 2500.0
                for h, base, step, mem in pool_ops:
                    t += len(mem) * HB * F * NS_COL_POOL + 150
                    ready[(mem[0], h)] = t

                # ---- emit product ops; record tiles ----
                ptile = {}   # (i, h) -> (tile, col0)
                for ops, pool_, eng in ((dve_ops, prod_pool, nc.vector),
                                        (pool_ops, pprod_pool, nc.gpsimd)):
                    for h, base, step, mem in ops:
                        ng = len(mem)
                        prod = pool_.tile(
                            [P, ng * HB * F], f16, name="prod",
                            tag=f"prod{eng.engine.value}",
                            padded_shape=[P, max_prod])
                        in0 = bass.AP(
                            X.tensor,
                            X.offset + base + h * HB * XPITCH,
                            [list(X.ap[0]), [step, ng], [XPITCH, HB], [1, F]],
                        )
                        c0 = dcol(mem[0])
                        in1 = bass.AP(
                            DS.tensor, DS.offset + c0,
                            [list(DS.ap[0]), [F, ng], [0, HB], [1, F]],
                        )
                        out4 = prod[:].rearrange("p (g b f) -> p g b f",
                                                 b=HB, f=F)
                        if KNOB_SWAP:
                            eng.tensor_mul(out4, in1, in0)
                        else:
                            eng.tensor_mul(out4, in0, in1)
                        for k, i in enumerate(mem):
                            ptile[(i, h)] = (prod, k * HB * F)

                # ---- PE matmuls in estimated-availability order ----
                order = sorted(ptile.keys(), key=lambda ih: ready[ih])
                seen = set()
                last = {}
                for b in range(BPC):
                    h = b // HB
                    last[b] = [ih for ih in order if ih[1] == h][-1]
                for i, h in order:
                    prod, c0 = ptile[(i, h)]
                    lhsT = DS[:, st0 + sidx[rots[i]] * P:
                              st0 + (sidx[rots[i]] + 1) * P]
                    for k in range(HB):
                        b = h * HB + k
                        nc.tensor.matmul(
                            psums[b][:], lhsT,
                            prod[:, c0 + k * F:c0 + (k + 1) * F],
                            start=(b not in seen),
                            stop=((i, h) == last[b]),
                        )
                        seen.add(b)

                # ---- eviction (Act) + output DMA per batch half ----
                for h in range(2):
                    ot = out_pool.tile([P, HB * F], f16, name=f"ot{h}",
                                       tag=f"ot{h}")
                    for k in range(HB):
                        b = h * HB + k
                        nc.scalar.copy(ot[:, k * F:(k + 1) * F], psums[b][:])
                    ydst = y[h * HB:(h + 1) * HB, :].rearrange(
                        "b (p f) -> p b f", f=F)
                    eng = nc.sync if h == 0 else nc.scalar
                    eng.dma_start(out=ydst, in_=ot[:].rearrange(
                        "p (b f) -> p b f", f=F))

            if repeat == 1:
                body()
            else:
                with tc.For_i(0, repeat, 1):
                    body()
    nc.finalize()
    return nc


def prep_inputs(x, diagonals, stride=1, reps=1):
    """Host-side shard + relayout. Returns in_maps for run_bass_kernel_spmd."""
    dec, rots, uniq, sidx, pool_idx, groups = _plan(stride, reps)
    ns = len(uniq)

    x16 = np.ascontiguousarray(x, dtype=np.float16)
    # halo tiles in SBUF layout: xh[p, b*XPITCH + j] = x[b, (512p + j) % S]
    j = np.arange(XPITCH)
    idx = (np.arange(P)[:, None] * F + j[None, :]) % SLOTS
    xt = x16[:, idx]                       # [BATCH, P, XPITCH]
    xh = np.ascontiguousarray(
        np.stack([np.transpose(xt[c * BPC:(c + 1) * BPC], (1, 0, 2))
                  .reshape(P, BPC * XPITCH) for c in range(N_CORES)])
    )

    # ds: [chunk1 diag blocks | stats | remaining diag blocks], with each
    # diag block pre-rotated along partitions by its shift's m.
    d16 = np.asarray(diagonals, dtype=np.float16).reshape(NUM_T, P, F)
    dd_order = list(groups[0][2]) + pool_idx[:1]
    for g in groups[1:]:
        dd_order += g[2]
    dd_order += pool_idx[1:]
    c1_blocks = len(groups[0][2]) + len(pool_idx[:1])
    ddl = [np.roll(d16[i], dec[i][0], axis=0) for i in dd_order]
    dd = np.transpose(np.stack(ddl), (1, 0, 2)).reshape(P, NUM_T * F)

    st = np.zeros((ns, P, P), np.float16)
    cols = np.arange(P)
    for k, a in enumerate(uniq):
        st[k, (cols + a) % P, cols] = 1.0
    st = np.transpose(st, (1, 0, 2)).reshape(P, ns * P)

    c1 = c1_blocks * F
    ds = np.ascontiguousarray(
        np.concatenate([dd[:, :c1], st, dd[:, c1:]], axis=1))

    in_maps = []
    for c in range(N_CORES):
        in_maps.append({"xh": xh[c], "ds": ds})
    return in_maps


_MODULE_CACHE = {}


def kernel(**inputs):
    x = np.asarray(inputs["x"], dtype=np.float32)
    diagonals = np.asarray(inputs["diagonals"], dtype=np.float32)
    stride = int(np.asarray(inputs.get("stride", 1)))
    reps = int(np.asarray(inputs.get("reps", 1)))
    assert x.shape == (BATCH, SLOTS) and diagonals.shape == (NUM_T, SLOTS)
    # halo must cover the largest in-partition shift
    dec, _, _, _ = _decomp(stride, reps)
    assert max(df for _, df in dec) <= HALO, "halo too small for this stride"

    key = (stride, reps)
    if key not in _MODULE_CACHE:
        _MODULE_CACHE[key] = build_module(stride, reps)
    nc = _MODULE_CACHE[key]

    in_maps = prep_inputs(x, diagonals, stride, reps)
    res = run_bass_kernel_spmd(nc, in_maps, list(range(N_CORES)))
    out = np.concatenate(
        [np.asarray(res.results[c]["y"]) for c in range(N_CORES)], axis=0
    )
    return out.astype(np.float32)


# revision 15
# speedup vs baseline: 1.5616x; 1.0322x over previous
"""Trainium2 Bass kernel for the FHE BSGS conv-as-linear-transform problem.

Computes, for each batch row b of x[64, 65536]:
    out[b, s] = sum_{i=0}^{15} x[b, (s + 2^i * stride) % S] * diagonals[i, s]
    out = roll(out, S // (2 * reps))            (S = 65536)

Distribution: batch dim sharded across 8 NeuronCores (8 rows per core),
diagonals + stationary matrices replicated. No cross-core communication.

Per-core algorithm (free-minor layout: slot s = 512*p + f, p = partition):
  - A shift delta = 512*m + df decomposes into a free-dim column offset df
    (read the x tile at offset df against a 256-column halo) and a partition
    rotation by m (folded into the accumulation matmul).
  - All 16 elementwise products run on DVE in f16 (2x mode), fused over all
    8 batches via a broadcast diagonal operand and over transform groups in
    arithmetic progression (max 3 per op so the PE feed stays smooth and
    prod-pool buffers stay small).  Ops are split by batch half so compute
    starts after half the x tile has landed, and each half's last op is a
    singleton to keep the pipeline tail short.  (The Pool engine measures
    fine in isolation but destroys the pipeline on real HW — SBUF port
    contention with DVE — so everything stays on DVE.)
  - TensorE accumulates every product tile into a per-batch PSUM bank via a
    128x128 rotation-permutation stationary matmul; the partition rotation
    of each shift AND the final roll (multiple of 512 slots) are folded in,
    so PSUM holds the final rolled output directly.  PE matmuls are emitted
    in estimated product-availability order (engines run in-order), after a
    burst of warmup matmuls on a zero tile that ride out the PE p-state
    ramp during the DMA prologue.
  - PSUM eviction downcasts to f16 (ScalarE for the first half, ScalarE +
    DVE in parallel for the last banks); y is stored f16 and upcast on
    host.  Output DMA per batch half on otherwise-idle queues.

All device-input arrays are prepared host-side in the exact SBUF layout so
every input DMA is a dense partition-major copy.
"""

import os
import sys

for _p in ("/opt/trn_rl_repo",):
    if os.path.isdir(_p) and _p not in sys.path:
        sys.path.insert(0, _p)

import numpy as np

import concourse.bass as bass
import concourse.mybir as mybir
from concourse import bacc
from concourse.bass_utils import run_bass_kernel_spmd
from concourse.tile import TileContext

N_CORES = 8
BATCH = 64
SLOTS = 65536
NUM_T = 16
P = 128                 # SBUF partitions
F = SLOTS // P          # 512 slots per partition per batch
BPC = BATCH // N_CORES  # 8 batch rows per core
HALO = 256              # halo columns (covers df <= 256 for stride=1)
XPITCH = F + HALO       # 768
MAX_GROUP = 3           # max transforms fused into one DVE op
NS_COL_DVE = 0.521      # DVE f16 2x ns/col (cost model)
OP_OVH = 160.0          # per-op overhead estimate (ns)
N_WARMUP = 8            # PE warmup matmuls (ride out p-state ramp)

KNOB_POOL = int(os.environ.get("K_POOL", "0"))    # transforms on Pool engine
KNOB_BUFS = int(os.environ.get("K_BUFS", "6"))    # prod pool buffers


def _decomp(stride, reps):
    """Per-transform (m_i, df_i) shift decomposition + stationary rotations."""
    roll = (SLOTS // (2 * reps)) % SLOTS
    assert roll % F == 0, f"final roll {roll} not a multiple of {F}"
    mr = roll // F
    dec = []
    for i in range(NUM_T):
        delta = ((1 << i) * stride) % SLOTS
        dec.append((delta // F, delta % F))
    rots = [(m - mr) % P for (m, _) in dec]
    uniq = sorted(set(rots))
    sidx = {a: j for j, a in enumerate(uniq)}
    return dec, rots, uniq, sidx


def _plan(stride, reps):
    """Group transforms into DVE ops and fix the execution order.

    Returns (dec, rots, uniq, sidx, pool_idx, groups); each group is
    (base, step, members) — a fused DVE op reading x at offsets
    base + k*step (all even except a lone odd singleton is allowed).
    Execution order: a singleton first (fast PE start), then pairs and
    3-member groups interleaved, ending with the remaining singletons so
    each half's tail op is short.
    """
    dec, rots, uniq, sidx = _decomp(stride, reps)
    pool = []
    if KNOB_POOL > 0:
        zeros = [i for i in range(NUM_T) if dec[i][1] == 0]
        while len(pool) < KNOB_POOL and zeros:
            pool.append(zeros.pop())
    rest = [i for i in range(NUM_T) if i not in pool]
    rows = sorted(rest, key=lambda i: dec[i][1])
    groups = []
    k = 0
    while k < len(rows):
        base = dec[rows[k]][1]
        members = [rows[k]]
        step = None
        j = k + 1
        while j < len(rows) and len(members) < MAX_GROUP:
            d = dec[rows[j]][1] - dec[rows[j - 1]][1]
            if d % 2 != 0 or base % 2 != 0:
                break
            if step is None:
                step = d
            elif d != step:
                break
            members.append(rows[j])
            j += 1
        groups.append((base, step or 0, members))
        k = j
    singles = [g for g in groups if len(g[2]) == 1]
    multis = sorted([g for g in groups if len(g[2]) > 1],
                    key=lambda g: len(g[2]))
    order = []
    if singles:
        order.append(singles.pop(0))
    # interleave small multis with big ones
    lo = [g for g in multis if len(g[2]) <= 2]
    hi = [g for g in multis if len(g[2]) > 2]
    while lo or hi:
        if lo:
            order.append(lo.pop(0))
        if hi:
            order.append(hi.pop(0))
    order += singles
    return dec, rots, uniq, sidx, pool, order


def build_module(stride=1, reps=1, debug=False, repeat=1):
    """Build + finalize the per-core Bass module (same program on all cores)."""
    dec, rots, uniq, sidx, pool_idx, groups = _plan(stride, reps)
    ns = len(uniq)
    HB = BPC // 2  # batches per half

    # ds column layout: [first-2-ops diags | stats | rest in consumption
    # order].  chunk1 (DVE queue) feeds the first two ops + the PE
    # stationaries; chunk2/chunk3 (SP queue, after x half1) feed the rest.
    dd_order = [i for (_, _, mem) in groups for i in mem] + pool_idx
    n_front = sum(len(groups[k][2]) for k in range(min(2, len(groups))))
    dd_col = {i: k for k, i in enumerate(dd_order)}
    st0 = n_front * F
    dcol = lambda i: (dd_col[i] * F if dd_col[i] < n_front
                      else dd_col[i] * F + ns * P)
    ds_cols = NUM_T * F + ns * P
    n_c1 = st0 + ns * P
    n_c2 = min(n_c1 + 5 * F, ds_cols)

    f16 = mybir.dt.float16
    f32 = mybir.dt.float32

    nc = bacc.Bacc("TRN2", target_bir_lowering=False, debug=debug,
                   num_devices=N_CORES)
    xh = nc.dram_tensor("xh", [P, BPC * XPITCH], f16, kind="ExternalInput")
    ds = nc.dram_tensor("ds", [P, ds_cols], f16, kind="ExternalInput")
    y = nc.dram_tensor("y", [BPC, SLOTS], f16, kind="ExternalOutput")

    with TileContext(nc) as tc:
        with (
            tc.tile_pool(name="xa", bufs=1) as xa_pool,
            tc.tile_pool(name="dda", bufs=1) as dd_pool,
            tc.tile_pool(name="wu", bufs=1) as wu_pool,
            tc.tile_pool(name="prod", bufs=KNOB_BUFS) as prod_pool,
            tc.tile_pool(name="pprod", bufs=2) as pprod_pool,
            tc.tile_pool(name="outs", bufs=2) as out_pool,
            tc.tile_pool(name="ps", bufs=1, space="PSUM") as ps_pool,
        ):
            DS = dd_pool.tile([P, ds_cols], f16)
            X = xa_pool.tile([P, BPC * XPITCH], f16)
            WU = wu_pool.tile([P, F], f16)
            xq = 2 * XPITCH
            xmid = HB * XPITCH
            # Pool queue (SWDGE): chunk1 (first diags + stats).  SP: x
            # batches 0-1 then 2-3 (the first product op covers batches
            # 0-1 only, so it starts after a quarter of x), then the
            # remaining diags.  Act: x half2 (behind the act-table load).
            nc.gpsimd.dma_start(out=DS[:, :n_c1], in_=ds[:, :n_c1])
            nc.sync.dma_start(out=X[:, :xq], in_=xh[:, :xq])
            nc.sync.dma_start(out=X[:, xq:xmid], in_=xh[:, xq:xmid])
            nc.scalar.dma_start(out=X[:, xmid:], in_=xh[:, xmid:])
            nc.sync.dma_start(out=DS[:, n_c1:n_c2], in_=ds[:, n_c1:n_c2])
            if n_c2 < ds_cols:
                nc.sync.dma_start(out=DS[:, n_c2:], in_=ds[:, n_c2:])
            nc.gpsimd.memset(WU[:], 0.0)

            max_prod = max(len(g[2]) for g in groups) * HB * F

            def body(_iv=None):
                psums = [
                    ps_pool.tile([P, F], f32, name=f"psum{b}", tag=f"ps{b}",
                                 bufs=1)
                    for b in range(BPC)
                ]
                # PE warmup: self-contained zero accumulation groups that
                # keep PE busy (and ramping) through the DMA prologue.
                for _ in range(N_WARMUP):
                    nc.tensor.matmul(psums[0][:], WU[:, :P], WU[:],
                                     start=True, stop=True)

                # op tuples: (b0, nb, base, step, mem).  Half split, except
                # the very first op (a singleton) runs as two batch-pair
                # quarters so compute starts after a quarter of x.
                dve_ops = []
                for h in (0, 1):
                    for gi, (base, step, mem) in enumerate(groups):
                        if h == 0 and gi == 0 and len(mem) == 1:
                            dve_ops.append((0, 2, base, step, mem))
                            dve_ops.append((2, 2, base, step, mem))
                        else:
                            dve_ops.append((h * HB, HB, base, step, mem))
                pool_ops = [(h * HB, HB, dec[i][1], 0, [i])
                            for h in (0, 1) for i in pool_idx]

                # estimated ready times (ns) for PE emission ordering
                ready = {}
                t = 2000.0
                for b0, nb, base, step, mem in dve_ops:
                    t += len(mem) * nb * F * NS_COL_DVE + OP_OVH
                    for i in mem:
                        ready[(i, b0)] = t
                t = 2500.0
                for b0, nb, base, step, mem in pool_ops:
                    t += len(mem) * nb * F * 1.99 + OP_OVH
                    ready[(mem[0], b0)] = t

                ptile = {}   # (i, b0) -> (tile, col0, nb)
                for ops, pool_, eng in ((dve_ops, prod_pool, nc.vector),
                                        (pool_ops, pprod_pool, nc.gpsimd)):
                    for b0, nb, base, step, mem in ops:
                        ng = len(mem)
                        prod = pool_.tile(
                            [P, ng * nb * F], f16, name="prod",
                            tag=f"prod{eng.engine.value}",
                            padded_shape=[P, max_prod])
                        in0 = bass.AP(
                            X.tensor,
                            X.offset + base + b0 * XPITCH,
                            [list(X.ap[0]), [step, ng], [XPITCH, nb], [1, F]],
                        )
                        c0 = dcol(mem[0])
                        in1 = bass.AP(
                            DS.tensor, DS.offset + c0,
                            [list(DS.ap[0]), [F, ng], [0, nb], [1, F]],
                        )
                        out4 = prod[:].rearrange("p (g b f) -> p g b f",
                                                 b=nb, f=F)
                        eng.tensor_mul(out4, in0, in1)
                        for k, i in enumerate(mem):
                            ptile[(i, b0)] = (prod, k * nb * F, nb)

                # PE matmuls in estimated-availability order
                order = sorted(ptile.keys(), key=lambda ib: ready[ib])
                seen = set()
                last = {}
                for b in range(BPC):
                    last[b] = [ib for ib in order
                               if ib[1] <= b < ib[1] + ptile[ib][2]][-1]
                for i, b0 in order:
                    prod, c0, nb = ptile[(i, b0)]
                    lhsT = DS[:, st0 + sidx[rots[i]] * P:
                              st0 + (sidx[rots[i]] + 1) * P]
                    for k in range(nb):
                        b = b0 + k
                        nc.tensor.matmul(
                            psums[b][:], lhsT,
                            prod[:, c0 + k * F:c0 + (k + 1) * F],
                            start=(b not in seen),
                            stop=((i, b0) == last[b]),
                        )
                        seen.add(b)

                # eviction + output DMA.  Half 1: all four banks on Act,
                # one y DMA on SP.  Half 2 (the tail): banks 4,5 on Act
                # and 6,7 on DVE in parallel, per-bank y DMAs on SP so
                # each bank ships as soon as its eviction lands.
                for h in range(2):
                    ot = out_pool.tile([P, HB * F], f16, name=f"ot{h}",
                                       tag=f"ot{h}")
                    for k in range(HB):
                        b = h * HB + k
                        dst = ot[:, k * F:(k + 1) * F]
                        if h == 1 and k >= 2:
                            nc.vector.tensor_copy(dst, psums[b][:])
                        else:
                            nc.scalar.copy(dst, psums[b][:])
                        if h == 1:
                            ydst = y[b:b + 1, :].rearrange(
                                "b (p f) -> p b f", f=F)
                            nc.sync.dma_start(
                                out=ydst,
                                in_=ot[:, k * F:(k + 1) * F].rearrange(
                                    "p (b f) -> p b f", f=F))
                    if h == 0:
                        ydst = y[:HB, :].rearrange("b (p f) -> p b f", f=F)
                        nc.sync.dma_start(out=ydst, in_=ot[:].rearrange(
                            "p (b f) -> p b f", f=F))

            if repeat == 1:
                body()
            else:
                with tc.For_i(0, repeat, 1):
                    body()
    nc.finalize()
    return nc


def prep_inputs(x, diagonals, stride=1, reps=1):
    """Host-side shard + relayout. Returns in_maps for run_bass_kernel_spmd."""
    dec, rots, uniq, sidx, pool_idx, groups = _plan(stride, reps)
    ns = len(uniq)

    x16 = np.ascontiguousarray(x, dtype=np.float16)
    # halo tiles in SBUF layout: xh[p, b*XPITCH + j] = x[b, (512p + j) % S]
    j = np.arange(XPITCH)
    idx = (np.arange(P)[:, None] * F + j[None, :]) % SLOTS
    xt = x16[:, idx]                       # [BATCH, P, XPITCH]
    xh = np.ascontiguousarray(
        np.stack([np.transpose(xt[c * BPC:(c + 1) * BPC], (1, 0, 2))
                  .reshape(P, BPC * XPITCH) for c in range(N_CORES)])
    )

    # ds: [first-2-ops diag blocks | stats | remaining diag blocks], with
    # each diag block pre-rotated along partitions by its shift's m.
    d16 = np.asarray(diagonals, dtype=np.float16).reshape(NUM_T, P, F)
    dd_order = [i for (_, _, mem) in groups for i in mem] + pool_idx
    n_front = sum(len(groups[k][2]) for k in range(min(2, len(groups))))
    ddl = [np.roll(d16[i], dec[i][0], axis=0) for i in dd_order]
    dd = np.transpose(np.stack(ddl), (1, 0, 2)).reshape(P, NUM_T * F)

    st = np.zeros((ns, P, P), np.float16)
    cols = np.arange(P)
    for k, a in enumerate(uniq):
        st[k, (cols + a) % P, cols] = 1.0
    st = np.transpose(st, (1, 0, 2)).reshape(P, ns * P)

    c1 = n_front * F
    ds = np.ascontiguousarray(
        np.concatenate([dd[:, :c1], st, dd[:, c1:]], axis=1))

    in_maps = []
    for c in range(N_CORES):
        in_maps.append({"xh": xh[c], "ds": ds})
    return in_maps


_MODULE_CACHE = {}


def kernel(**inputs):
    x = np.asarray(inputs["x"], dtype=np.float32)
    diagonals = np.asarray(inputs["diagonals"], dtype=np.float32)
    stride = int(np.asarray(inputs.get("stride", 1)))
    reps = int(np.asarray(inputs.get("reps", 1)))
    assert x.shape == (BATCH, SLOTS) and diagonals.shape == (NUM_T, SLOTS)
    # halo must cover the largest in-partition shift
    dec, _, _, _ = _decomp(stride, reps)
    assert max(df for _, df in dec) <= HALO, "halo too small for this stride"

    key = (stride, reps)
    if key not in _MODULE_CACHE:
        _MODULE_CACHE[key] = build_module(stride, reps)
    nc = _MODULE_CACHE[key]

    in_maps = prep_inputs(x, diagonals, stride, reps)
    res = run_bass_kernel_spmd(nc, in_maps, list(range(N_CORES)))
    out = np.concatenate(
        [np.asarray(res.results[c]["y"]) for c in range(N_CORES)], axis=0
    )
    return out.astype(np.float32)


# revision 17
# speedup vs baseline: 1.5686x; 1.0044x over previous
"""Trainium2 Bass kernel for the FHE BSGS conv-as-linear-transform problem.

Computes, for each batch row b of x[64, 65536]:
    out[b, s] = sum_{i=0}^{15} x[b, (s + 2^i * stride) % S] * diagonals[i, s]
    out = roll(out, S // (2 * reps))            (S = 65536)

Distribution: batch dim sharded across 8 NeuronCores (8 rows per core),
diagonals + stationary matrices replicated. No cross-core communication.

Per-core algorithm (free-minor layout: slot s = 512*p + f, p = partition):
  - A shift delta = 512*m + df decomposes into a free-dim column offset df
    (read the x tile at offset df against a 256-column halo) and a partition
    rotation by m (folded into the accumulation matmul).
  - All 16 elementwise products run on DVE in f16 (2x mode), fused over all
    8 batches via a broadcast diagonal operand and over transform groups in
    arithmetic progression (max 3 per op so the PE feed stays smooth and
    prod-pool buffers stay small).  Ops are split by batch half so compute
    starts after half the x tile has landed, and each half's last op is a
    singleton to keep the pipeline tail short.  (The Pool engine measures
    fine in isolation but destroys the pipeline on real HW — SBUF port
    contention with DVE — so everything stays on DVE.)
  - TensorE accumulates every product tile into a per-batch PSUM bank via a
    128x128 rotation-permutation stationary matmul; the partition rotation
    of each shift AND the final roll (multiple of 512 slots) are folded in,
    so PSUM holds the final rolled output directly.  PE matmuls are emitted
    in estimated product-availability order (engines run in-order), after a
    burst of warmup matmuls on a zero tile that ride out the PE p-state
    ramp during the DMA prologue.
  - PSUM eviction downcasts to f16 (ScalarE for the first half, ScalarE +
    DVE in parallel for the last banks); y is stored f16 and upcast on
    host.  Output DMA per batch half on otherwise-idle queues.

All device-input arrays are prepared host-side in the exact SBUF layout so
every input DMA is a dense partition-major copy.
"""

import os
import sys

for _p in ("/opt/trn_rl_repo",):
    if os.path.isdir(_p) and _p not in sys.path:
        sys.path.insert(0, _p)

import numpy as np

import concourse.bass as bass
import concourse.mybir as mybir
from concourse import bacc
from concourse.bass_utils import run_bass_kernel_spmd
from concourse.tile import TileContext

N_CORES = 8
BATCH = 64
SLOTS = 65536
NUM_T = 16
P = 128                 # SBUF partitions
F = SLOTS // P          # 512 slots per partition per batch
BPC = BATCH // N_CORES  # 8 batch rows per core
HALO = 256              # halo columns (covers df <= 256 for stride=1)
XPITCH = F + HALO       # 768
MAX_GROUP = 3           # max transforms fused into one DVE op
NS_COL_DVE = 0.521      # DVE f16 2x ns/col (cost model)
OP_OVH = 160.0          # per-op overhead estimate (ns)
N_WARMUP = 8            # PE warmup matmuls (ride out p-state ramp)

KNOB_POOL = int(os.environ.get("K_POOL", "0"))    # transforms on Pool engine
KNOB_BUFS = int(os.environ.get("K_BUFS", "6"))    # prod pool buffers


def _decomp(stride, reps):
    """Per-transform (m_i, df_i) shift decomposition + stationary rotations."""
    roll = (SLOTS // (2 * reps)) % SLOTS
    assert roll % F == 0, f"final roll {roll} not a multiple of {F}"
    mr = roll // F
    dec = []
    for i in range(NUM_T):
        delta = ((1 << i) * stride) % SLOTS
        dec.append((delta // F, delta % F))
    rots = [(m - mr) % P for (m, _) in dec]
    uniq = sorted(set(rots))
    sidx = {a: j for j, a in enumerate(uniq)}
    return dec, rots, uniq, sidx


def _plan(stride, reps):
    """Group transforms into DVE ops and fix the execution order.

    Returns (dec, rots, uniq, sidx, pool_idx, groups); each group is
    (base, step, members) — a fused DVE op reading x at offsets
    base + k*step (all even except a lone odd singleton is allowed).
    Execution order: a singleton first (fast PE start), then pairs and
    3-member groups interleaved, ending with the remaining singletons so
    each half's tail op is short.
    """
    dec, rots, uniq, sidx = _decomp(stride, reps)
    pool = []
    if KNOB_POOL > 0:
        zeros = [i for i in range(NUM_T) if dec[i][1] == 0]
        while len(pool) < KNOB_POOL and zeros:
            pool.append(zeros.pop())
    rest = [i for i in range(NUM_T) if i not in pool]
    rows = sorted(rest, key=lambda i: dec[i][1])
    groups = []
    k = 0
    while k < len(rows):
        base = dec[rows[k]][1]
        members = [rows[k]]
        step = None
        j = k + 1
        while j < len(rows) and len(members) < MAX_GROUP:
            d = dec[rows[j]][1] - dec[rows[j - 1]][1]
            if d % 2 != 0 or base % 2 != 0:
                break
            if step is None:
                step = d
            elif d != step:
                break
            members.append(rows[j])
            j += 1
        groups.append((base, step or 0, members))
        k = j
    singles = [g for g in groups if len(g[2]) == 1]
    multis = sorted([g for g in groups if len(g[2]) > 1],
                    key=lambda g: len(g[2]))
    order = []
    if singles:
        order.append(singles.pop(0))
    # interleave small multis with big ones
    lo = [g for g in multis if len(g[2]) <= 2]
    hi = [g for g in multis if len(g[2]) > 2]
    while lo or hi:
        if lo:
            order.append(lo.pop(0))
        if hi:
            order.append(hi.pop(0))
    order += singles
    return dec, rots, uniq, sidx, pool, order


def build_module(stride=1, reps=1, debug=False, repeat=1):
    """Build + finalize the per-core Bass module (same program on all cores)."""
    dec, rots, uniq, sidx, pool_idx, groups = _plan(stride, reps)
    ns = len(uniq)
    HB = BPC // 2  # batches per half

    # ds column layout: [first-2-ops diags | stats | rest in consumption
    # order].  chunk1 (DVE queue) feeds the first two ops + the PE
    # stationaries; chunk2/chunk3 (SP queue, after x half1) feed the rest.
    dd_order = [i for (_, _, mem) in groups for i in mem] + pool_idx
    n_front = sum(len(groups[k][2]) for k in range(min(2, len(groups))))
    dd_col = {i: k for k, i in enumerate(dd_order)}
    st0 = n_front * F
    dcol = lambda i: (dd_col[i] * F if dd_col[i] < n_front
                      else dd_col[i] * F + ns * P)
    ds_cols = NUM_T * F + ns * P
    n_c1 = st0 + ns * P
    n_c2 = min(n_c1 + 5 * F, ds_cols)

    f16 = mybir.dt.float16
    f32 = mybir.dt.float32

    nc = bacc.Bacc("TRN2", target_bir_lowering=False, debug=debug,
                   num_devices=N_CORES)
    xh = nc.dram_tensor("xh", [P, BPC * XPITCH], f16, kind="ExternalInput")
    ds = nc.dram_tensor("ds", [P, ds_cols], f16, kind="ExternalInput")
    y = nc.dram_tensor("y", [BPC, SLOTS], f16, kind="ExternalOutput")

    with TileContext(nc) as tc:
        with (
            tc.tile_pool(name="xa", bufs=1) as xa_pool,
            tc.tile_pool(name="dda", bufs=1) as dd_pool,
            tc.tile_pool(name="wu", bufs=1) as wu_pool,
            tc.tile_pool(name="prod", bufs=KNOB_BUFS) as prod_pool,
            tc.tile_pool(name="pprod", bufs=2) as pprod_pool,
            tc.tile_pool(name="outs", bufs=2) as out_pool,
            tc.tile_pool(name="ps", bufs=1, space="PSUM") as ps_pool,
        ):
            DS = dd_pool.tile([P, ds_cols], f16)
            X = xa_pool.tile([P, BPC * XPITCH], f16)
            WU = wu_pool.tile([P, F], f16)
            xq = 2 * XPITCH
            xmid = HB * XPITCH
            # Pool queue (SWDGE): first-2-ops diags, then the stats (the
            # stats only matter once warmup matmuls run out).  SP: x
            # batches 0-1 then 2-3 (the first product op covers batches
            # 0-1 only, so it starts after a quarter of x), then the
            # remaining diags.  Act: x half2 (behind the act-table load).
            nc.gpsimd.dma_start(out=DS[:, :st0], in_=ds[:, :st0])
            nc.gpsimd.dma_start(out=DS[:, st0:n_c1], in_=ds[:, st0:n_c1])
            nc.sync.dma_start(out=X[:, :xq], in_=xh[:, :xq])
            nc.sync.dma_start(out=X[:, xq:xmid], in_=xh[:, xq:xmid])
            nc.scalar.dma_start(out=X[:, xmid:], in_=xh[:, xmid:])
            nc.sync.dma_start(out=DS[:, n_c1:n_c2], in_=ds[:, n_c1:n_c2])
            if n_c2 < ds_cols:
                nc.sync.dma_start(out=DS[:, n_c2:], in_=ds[:, n_c2:])
            nc.gpsimd.memset(WU[:], 0.0)

            max_prod = max(len(g[2]) for g in groups) * HB * F

            def body(_iv=None):
                psums = [
                    ps_pool.tile([P, F], f32, name=f"psum{b}", tag=f"ps{b}",
                                 bufs=1)
                    for b in range(BPC)
                ]
                # PE warmup: self-contained zero accumulation groups that
                # keep PE busy (and ramping) through the DMA prologue.
                for _ in range(N_WARMUP):
                    nc.tensor.matmul(psums[0][:], WU[:, :P], WU[:],
                                     start=True, stop=True)

                # op tuples: (b0, nb, base, step, mem).  Half split, except
                # the very first op (a singleton) runs as two batch-pair
                # quarters so compute starts after a quarter of x.
                dve_ops = []
                for h in (0, 1):
                    for gi, (base, step, mem) in enumerate(groups):
                        if h == 0 and gi == 0 and len(mem) == 1:
                            dve_ops.append((0, 2, base, step, mem))
                            dve_ops.append((2, 2, base, step, mem))
                        else:
                            dve_ops.append((h * HB, HB, base, step, mem))
                pool_ops = [(h * HB, HB, dec[i][1], 0, [i])
                            for h in (0, 1) for i in pool_idx]

                # estimated ready times (ns) for PE emission ordering
                ready = {}
                t = 2000.0
                for b0, nb, base, step, mem in dve_ops:
                    t += len(mem) * nb * F * NS_COL_DVE + OP_OVH
                    for i in mem:
                        ready[(i, b0)] = t
                t = 2500.0
                for b0, nb, base, step, mem in pool_ops:
                    t += len(mem) * nb * F * 1.99 + OP_OVH
                    ready[(mem[0], b0)] = t

                ptile = {}   # (i, b0) -> (tile, col0, nb)
                for ops, pool_, eng in ((dve_ops, prod_pool, nc.vector),
                                        (pool_ops, pprod_pool, nc.gpsimd)):
                    for b0, nb, base, step, mem in ops:
                        ng = len(mem)
                        prod = pool_.tile(
                            [P, ng * nb * F], f16, name="prod",
                            tag=f"prod{eng.engine.value}",
                            padded_shape=[P, max_prod])
                        in0 = bass.AP(
                            X.tensor,
                            X.offset + base + b0 * XPITCH,
                            [list(X.ap[0]), [step, ng], [XPITCH, nb], [1, F]],
                        )
                        c0 = dcol(mem[0])
                        in1 = bass.AP(
                            DS.tensor, DS.offset + c0,
                            [list(DS.ap[0]), [F, ng], [0, nb], [1, F]],
                        )
                        out4 = prod[:].rearrange("p (g b f) -> p g b f",
                                                 b=nb, f=F)
                        eng.tensor_mul(out4, in0, in1)
                        for k, i in enumerate(mem):
                            ptile[(i, b0)] = (prod, k * nb * F, nb)

                # PE matmuls in estimated-availability order
                order = sorted(ptile.keys(), key=lambda ib: ready[ib])
                seen = set()
                last = {}
                for b in range(BPC):
                    last[b] = [ib for ib in order
                               if ib[1] <= b < ib[1] + ptile[ib][2]][-1]
                for i, b0 in order:
                    prod, c0, nb = ptile[(i, b0)]
                    lhsT = DS[:, st0 + sidx[rots[i]] * P:
                              st0 + (sidx[rots[i]] + 1) * P]
                    for k in range(nb):
                        b = b0 + k
                        nc.tensor.matmul(
                            psums[b][:], lhsT,
                            prod[:, c0 + k * F:c0 + (k + 1) * F],
                            start=(b not in seen),
                            stop=((i, b0) == last[b]),
                        )
                        seen.add(b)

                # eviction + output DMA.  Half 1: all four banks on Act,
                # one y DMA on SP.  Half 2 (the tail): banks 4,5 on Act
                # and 6,7 on DVE in parallel, per-bank y DMAs on SP so
                # each bank ships as soon as its eviction lands.
                for h in range(2):
                    ot = out_pool.tile([P, HB * F], f16, name=f"ot{h}",
                                       tag=f"ot{h}")
                    for k in range(HB):
                        b = h * HB + k
                        dst = ot[:, k * F:(k + 1) * F]
                        if h == 1 and k >= 2:
                            nc.vector.tensor_copy(dst, psums[b][:])
                        else:
                            nc.scalar.copy(dst, psums[b][:])
                        if h == 1:
                            ydst = y[b:b + 1, :].rearrange(
                                "b (p f) -> p b f", f=F)
                            # banks 4,5 ship via Act's queue, 6,7 via SP
                            qeng = nc.scalar if k < 2 else nc.sync
                            qeng.dma_start(
                                out=ydst,
                                in_=ot[:, k * F:(k + 1) * F].rearrange(
                                    "p (b f) -> p b f", f=F))
                    if h == 0:
                        ydst = y[:HB, :].rearrange("b (p f) -> p b f", f=F)
                        nc.sync.dma_start(out=ydst, in_=ot[:].rearrange(
                            "p (b f) -> p b f", f=F))

            if repeat == 1:
                body()
            else:
                with tc.For_i(0, repeat, 1):
                    body()
    nc.finalize()
    return nc


def prep_inputs(x, diagonals, stride=1, reps=1):
    """Host-side shard + relayout. Returns in_maps for run_bass_kernel_spmd."""
    dec, rots, uniq, sidx, pool_idx, groups = _plan(stride, reps)
    ns = len(uniq)

    x16 = np.ascontiguousarray(x, dtype=np.float16)
    # halo tiles in SBUF layout: xh[p, b*XPITCH + j] = x[b, (512p + j) % S]
    j = np.arange(XPITCH)
    idx = (np.arange(P)[:, None] * F + j[None, :]) % SLOTS
    xt = x16[:, idx]                       # [BATCH, P, XPITCH]
    xh = np.ascontiguousarray(
        np.stack([np.transpose(xt[c * BPC:(c + 1) * BPC], (1, 0, 2))
                  .reshape(P, BPC * XPITCH) for c in range(N_CORES)])
    )

    # ds: [first-2-ops diag blocks | stats | remaining diag blocks], with
    # each diag block pre-rotated along partitions by its shift's m.
    d16 = np.asarray(diagonals, dtype=np.float16).reshape(NUM_T, P, F)
    dd_order = [i for (_, _, mem) in groups for i in mem] + pool_idx
    n_front = sum(len(groups[k][2]) for k in range(min(2, len(groups))))
    ddl = [np.roll(d16[i], dec[i][0], axis=0) for i in dd_order]
    dd = np.transpose(np.stack(ddl), (1, 0, 2)).reshape(P, NUM_T * F)

    st = np.zeros((ns, P, P), np.float16)
    cols = np.arange(P)
    for k, a in enumerate(uniq):
        st[k, (cols + a) % P, cols] = 1.0
    st = np.transpose(st, (1, 0, 2)).reshape(P, ns * P)

    c1 = n_front * F
    ds = np.ascontiguousarray(
        np.concatenate([dd[:, :c1], st, dd[:, c1:]], axis=1))

    in_maps = []
    for c in range(N_CORES):
        in_maps.append({"xh": xh[c], "ds": ds})
    return in_maps


_MODULE_CACHE = {}


def kernel(**inputs):
    x = np.asarray(inputs["x"], dtype=np.float32)
    diagonals = np.asarray(inputs["diagonals"], dtype=np.float32)
    stride = int(np.asarray(inputs.get("stride", 1)))
    reps = int(np.asarray(inputs.get("reps", 1)))
    assert x.shape == (BATCH, SLOTS) and diagonals.shape == (NUM_T, SLOTS)
    # halo must cover the largest in-partition shift
    dec, _, _, _ = _decomp(stride, reps)
    assert max(df for _, df in dec) <= HALO, "halo too small for this stride"

    key = (stride, reps)
    if key not in _MODULE_CACHE:
        _MODULE_CACHE[key] = build_module(stride, reps)
    nc = _MODULE_CACHE[key]

    in_maps = prep_inputs(x, diagonals, stride, reps)
    res = run_bass_kernel_spmd(nc, in_maps, list(range(N_CORES)))
    out = np.concatenate(
        [np.asarray(res.results[c]["y"]) for c in range(N_CORES)], axis=0
    )
    return out.astype(np.float32)


# revision 19
# speedup vs baseline: 1.6950x; 1.0806x over previous
"""Trainium2 Bass kernel for the FHE BSGS conv-as-linear-transform problem.

Computes, for each batch row b of x[64, 65536]:
    out[b, s] = sum_{i=0}^{15} x[b, (s + 2^i * stride) % S] * diagonals[i, s]
    out = roll(out, S // (2 * reps))            (S = 65536)

Distribution: batch dim sharded across 8 NeuronCores (8 rows per core),
diagonals + stationary matrices replicated. No cross-core communication.

Per-core algorithm (free-minor layout: slot s = 512*p + f, p = partition):
  - A shift delta = 512*m + df decomposes into a free-dim column offset df
    (read the x tile at offset df against a 256-column halo) and a partition
    rotation by m (folded into the accumulation matmul).
  - All 16 elementwise products run on DVE in f16 (2x mode), fused over all
    8 batches via a broadcast diagonal operand and over transform groups in
    arithmetic progression (max 3 per op so the PE feed stays smooth and
    prod-pool buffers stay small).  Ops are split by batch half so compute
    starts after half the x tile has landed, and each half's last op is a
    singleton to keep the pipeline tail short.  (The Pool engine measures
    fine in isolation but destroys the pipeline on real HW — SBUF port
    contention with DVE — so everything stays on DVE.)
  - TensorE accumulates every product tile into a per-batch PSUM bank via a
    128x128 rotation-permutation stationary matmul; the partition rotation
    of each shift AND the final roll (multiple of 512 slots) are folded in,
    so PSUM holds the final rolled output directly.  PE matmuls are emitted
    in estimated product-availability order (engines run in-order), after a
    burst of warmup matmuls on a zero tile that ride out the PE p-state
    ramp during the DMA prologue.
  - PSUM eviction downcasts to f16 (ScalarE for the first half, ScalarE +
    DVE in parallel for the last banks); y is stored f16 and upcast on
    host.  Output DMA per batch half on otherwise-idle queues.

All device-input arrays are prepared host-side in the exact SBUF layout so
every input DMA is a dense partition-major copy.
"""

import os
import sys

for _p in ("/opt/trn_rl_repo",):
    if os.path.isdir(_p) and _p not in sys.path:
        sys.path.insert(0, _p)

import numpy as np

import concourse.bass as bass
import concourse.mybir as mybir
from concourse import bacc
from concourse.bass_utils import run_bass_kernel_spmd
from concourse.tile import TileContext

N_CORES = 8
BATCH = 64
SLOTS = 65536
NUM_T = 16
P = 128                 # SBUF partitions
F = SLOTS // P          # 512 slots per partition per batch
BPC = BATCH // N_CORES  # 8 batch rows per core
HALO = 256              # halo columns (covers df <= 256 for stride=1)
XPITCH = F + HALO       # 768
MAX_GROUP = 3           # max transforms fused into one DVE op
NS_COL_DVE = 0.521      # DVE f16 2x ns/col (cost model)
OP_OVH = 160.0          # per-op overhead estimate (ns)
N_WARMUP = 8            # PE warmup matmuls (ride out p-state ramp)

KNOB_POOL = int(os.environ.get("K_POOL", "0"))    # transforms on Pool engine
KNOB_BUFS = int(os.environ.get("K_BUFS", "6"))    # prod pool buffers


def _decomp(stride, reps):
    """Per-transform (m_i, df_i) shift decomposition + stationary rotations."""
    roll = (SLOTS // (2 * reps)) % SLOTS
    assert roll % F == 0, f"final roll {roll} not a multiple of {F}"
    mr = roll // F
    dec = []
    for i in range(NUM_T):
        delta = ((1 << i) * stride) % SLOTS
        dec.append((delta // F, delta % F))
    rots = [(m - mr) % P for (m, _) in dec]
    uniq = sorted(set(rots))
    sidx = {a: j for j, a in enumerate(uniq)}
    return dec, rots, uniq, sidx


def _plan(stride, reps):
    """Group transforms into DVE ops and fix the execution order.

    Returns (dec, rots, uniq, sidx, pool_idx, groups); each group is
    (base, step, members) — a fused DVE op reading x at offsets
    base + k*step (all even except a lone odd singleton is allowed).
    Execution order: a singleton first (fast PE start), then pairs and
    3-member groups interleaved, ending with the remaining singletons so
    each half's tail op is short.
    """
    dec, rots, uniq, sidx = _decomp(stride, reps)
    pool = []
    if KNOB_POOL > 0:
        zeros = [i for i in range(NUM_T) if dec[i][1] == 0]
        while len(pool) < KNOB_POOL and zeros:
            pool.append(zeros.pop())
    rest = [i for i in range(NUM_T) if i not in pool]
    rows = sorted(rest, key=lambda i: dec[i][1])
    groups = []
    k = 0
    while k < len(rows):
        base = dec[rows[k]][1]
        members = [rows[k]]
        step = None
        j = k + 1
        while j < len(rows) and len(members) < MAX_GROUP:
            d = dec[rows[j]][1] - dec[rows[j - 1]][1]
            if d % 2 != 0 or base % 2 != 0:
                break
            if step is None:
                step = d
            elif d != step:
                break
            members.append(rows[j])
            j += 1
        groups.append((base, step or 0, members))
        k = j
    singles = [g for g in groups if len(g[2]) == 1]
    multis = sorted([g for g in groups if len(g[2]) > 1],
                    key=lambda g: len(g[2]))
    order = []
    if singles:
        order.append(singles.pop(0))
    # interleave small multis with big ones
    lo = [g for g in multis if len(g[2]) <= 2]
    hi = [g for g in multis if len(g[2]) > 2]
    while lo or hi:
        if lo:
            order.append(lo.pop(0))
        if hi:
            order.append(hi.pop(0))
    order += singles
    return dec, rots, uniq, sidx, pool, order


def build_module(stride=1, reps=1, debug=False, repeat=1):
    """Build + finalize the per-core Bass module (same program on all cores)."""
    dec, rots, uniq, sidx, pool_idx, groups = _plan(stride, reps)
    ns = len(uniq)
    HB = BPC // 2  # batches per half

    # ds column layout: [first-2-ops diags | stats | rest in consumption
    # order].  chunk1 (DVE queue) feeds the first two ops + the PE
    # stationaries; chunk2/chunk3 (SP queue, after x half1) feed the rest.
    dd_order = [i for (_, _, mem) in groups for i in mem] + pool_idx
    n_front = sum(len(groups[k][2]) for k in range(min(2, len(groups))))
    dd_col = {i: k for k, i in enumerate(dd_order)}
    st0 = n_front * F
    dcol = lambda i: (dd_col[i] * F if dd_col[i] < n_front
                      else dd_col[i] * F + ns * P)
    ds_cols = NUM_T * F + ns * P
    n_c1 = st0 + ns * P
    n_c2 = min(n_c1 + 5 * F, ds_cols)

    f16 = mybir.dt.float16
    f32 = mybir.dt.float32

    nc = bacc.Bacc("TRN2", target_bir_lowering=False, debug=debug,
                   num_devices=N_CORES)
    xh = nc.dram_tensor("xh", [P, BPC * XPITCH], f16, kind="ExternalInput")
    ds = nc.dram_tensor("ds", [P, ds_cols], f16, kind="ExternalInput")
    y = nc.dram_tensor("y", [BPC, SLOTS], f16, kind="ExternalOutput")

    with TileContext(nc) as tc:
        with (
            tc.tile_pool(name="xa", bufs=1) as xa_pool,
            tc.tile_pool(name="dda", bufs=1) as dd_pool,
            tc.tile_pool(name="wu", bufs=1) as wu_pool,
            tc.tile_pool(name="prod", bufs=KNOB_BUFS) as prod_pool,
            tc.tile_pool(name="pprod", bufs=2) as pprod_pool,
            tc.tile_pool(name="outs", bufs=2) as out_pool,
            tc.tile_pool(name="ps", bufs=1, space="PSUM") as ps_pool,
        ):
            DS = dd_pool.tile([P, ds_cols], f16)
            X = xa_pool.tile([P, BPC * XPITCH], f16)
            WU = wu_pool.tile([P, F], f16)
            xq = 2 * XPITCH
            xmid = HB * XPITCH
            # Pool queue (SWDGE): chunk1 (first-2-ops diags + stats).  SP:
            # x batches 0-1 then 2-3 (the first product op covers batches
            # 0-1 only, so it starts after a quarter of x), then the
            # remaining diags.  Act: x half2 (behind the act-table load).
            nc.gpsimd.dma_start(out=DS[:, :n_c1], in_=ds[:, :n_c1])
            nc.sync.dma_start(out=X[:, :xq], in_=xh[:, :xq])
            nc.sync.dma_start(out=X[:, xq:xmid], in_=xh[:, xq:xmid])
            nc.scalar.dma_start(out=X[:, xmid:], in_=xh[:, xmid:])
            nc.sync.dma_start(out=DS[:, n_c1:n_c2], in_=ds[:, n_c1:n_c2])
            if n_c2 < ds_cols:
                nc.sync.dma_start(out=DS[:, n_c2:], in_=ds[:, n_c2:])
            nc.gpsimd.memset(WU[:], 0.0)

            max_prod = max(len(g[2]) for g in groups) * HB * F

            def body(_iv=None):
                psums = [
                    ps_pool.tile([P, F], f32, name=f"psum{b}", tag=f"ps{b}",
                                 bufs=1)
                    for b in range(BPC)
                ]
                # PE warmup: self-contained zero accumulation groups that
                # keep PE busy (and ramping) through the DMA prologue.
                for _ in range(N_WARMUP):
                    nc.tensor.matmul(psums[0][:], WU[:, :P], WU[:],
                                     start=True, stop=True)

                # op tuples: (b0, nb, base, step, mem).  Half split, except
                # the very first op (a singleton) runs as two batch-pair
                # quarters so compute starts after a quarter of x.
                dve_ops = []
                for h in (0, 1):
                    for gi, (base, step, mem) in enumerate(groups):
                        if h == 0 and gi == 0 and len(mem) == 1:
                            dve_ops.append((0, 2, base, step, mem))
                            dve_ops.append((2, 2, base, step, mem))
                        else:
                            dve_ops.append((h * HB, HB, base, step, mem))
                pool_ops = [(h * HB, HB, dec[i][1], 0, [i])
                            for h in (0, 1) for i in pool_idx]

                # estimated ready times (ns) for PE emission ordering
                ready = {}
                t = 2000.0
                for b0, nb, base, step, mem in dve_ops:
                    t += len(mem) * nb * F * NS_COL_DVE + OP_OVH
                    for i in mem:
                        ready[(i, b0)] = t
                t = 2500.0
                for b0, nb, base, step, mem in pool_ops:
                    t += len(mem) * nb * F * 1.99 + OP_OVH
                    ready[(mem[0], b0)] = t

                ptile = {}   # (i, b0) -> (tile, col0, nb)
                for ops, pool_, eng in ((dve_ops, prod_pool, nc.vector),
                                        (pool_ops, pprod_pool, nc.gpsimd)):
                    for b0, nb, base, step, mem in ops:
                        ng = len(mem)
                        prod = pool_.tile(
                            [P, ng * nb * F], f16, name="prod",
                            tag=f"prod{eng.engine.value}",
                            padded_shape=[P, max_prod])
                        in0 = bass.AP(
                            X.tensor,
                            X.offset + base + b0 * XPITCH,
                            [list(X.ap[0]), [step, ng], [XPITCH, nb], [1, F]],
                        )
                        c0 = dcol(mem[0])
                        in1 = bass.AP(
                            DS.tensor, DS.offset + c0,
                            [list(DS.ap[0]), [F, ng], [0, nb], [1, F]],
                        )
                        out4 = prod[:].rearrange("p (g b f) -> p g b f",
                                                 b=nb, f=F)
                        eng.tensor_mul(out4, in0, in1)
                        for k, i in enumerate(mem):
                            ptile[(i, b0)] = (prod, k * nb * F, nb)

                # PE matmuls in estimated-availability order
                order = sorted(ptile.keys(), key=lambda ib: ready[ib])
                seen = set()
                last = {}
                for b in range(BPC):
                    last[b] = [ib for ib in order
                               if ib[1] <= b < ib[1] + ptile[ib][2]][-1]
                for i, b0 in order:
                    prod, c0, nb = ptile[(i, b0)]
                    lhsT = DS[:, st0 + sidx[rots[i]] * P:
                              st0 + (sidx[rots[i]] + 1) * P]
                    for k in range(nb):
                        b = b0 + k
                        nc.tensor.matmul(
                            psums[b][:], lhsT,
                            prod[:, c0 + k * F:c0 + (k + 1) * F],
                            start=(b not in seen),
                            stop=((i, b0) == last[b]),
                        )
                        seen.add(b)

                # eviction + output DMA.  Half 1: all four banks on Act,
                # one y DMA on SP.  Half 2 (the tail): banks 4,5 on Act
                # and 6,7 on DVE in parallel, per-bank y DMAs on SP so
                # each bank ships as soon as its eviction lands.
                for h in range(2):
                    ot = out_pool.tile([P, HB * F], f16, name=f"ot{h}",
                                       tag=f"ot{h}")
                    for k in range(HB):
                        b = h * HB + k
                        dst = ot[:, k * F:(k + 1) * F]
                        if h == 1 and k >= 2:
                            nc.vector.tensor_copy(dst, psums[b][:])
                        else:
                            nc.scalar.copy(dst, psums[b][:])
                        if h == 1:
                            ydst = y[b:b + 1, :].rearrange(
                                "b (p f) -> p b f", f=F)
                            nc.sync.dma_start(
                                out=ydst,
                                in_=ot[:, k * F:(k + 1) * F].rearrange(
                                    "p (b f) -> p b f", f=F))
                    if h == 0:
                        ydst = y[:HB, :].rearrange("b (p f) -> p b f", f=F)
                        nc.sync.dma_start(out=ydst, in_=ot[:].rearrange(
                            "p (b f) -> p b f", f=F))

            if repeat == 1:
                body()
            else:
                with tc.For_i(0, repeat, 1):
                    body()
    nc.finalize()
    return nc


def prep_inputs(x, diagonals, stride=1, reps=1):
    """Host-side shard + relayout. Returns in_maps for run_bass_kernel_spmd."""
    dec, rots, uniq, sidx, pool_idx, groups = _plan(stride, reps)
    ns = len(uniq)

    x16 = np.ascontiguousarray(x, dtype=np.float16)
    # halo tiles in SBUF layout: xh[p, b*XPITCH + j] = x[b, (512p + j) % S]
    j = np.arange(XPITCH)
    idx = (np.arange(P)[:, None] * F + j[None, :]) % SLOTS
    xt = x16[:, idx]                       # [BATCH, P, XPITCH]
    xh = np.ascontiguousarray(
        np.stack([np.transpose(xt[c * BPC:(c + 1) * BPC], (1, 0, 2))
                  .reshape(P, BPC * XPITCH) for c in range(N_CORES)])
    )

    # ds: [first-2-ops diag blocks | stats | remaining diag blocks], with
    # each diag block pre-rotated along partitions by its shift's m.
    d16 = np.asarray(diagonals, dtype=np.float16).reshape(NUM_T, P, F)
    dd_order = [i for (_, _, mem) in groups for i in mem] + pool_idx
    n_front = sum(len(groups[k][2]) for k in range(min(2, len(groups))))
    ddl = [np.roll(d16[i], dec[i][0], axis=0) for i in dd_order]
    dd = np.transpose(np.stack(ddl), (1, 0, 2)).reshape(P, NUM_T * F)

    st = np.zeros((ns, P, P), np.float16)
    cols = np.arange(P)
    for k, a in enumerate(uniq):
        st[k, (cols + a) % P, cols] = 1.0
    st = np.transpose(st, (1, 0, 2)).reshape(P, ns * P)

    c1 = n_front * F
    ds = np.ascontiguousarray(
        np.concatenate([dd[:, :c1], st, dd[:, c1:]], axis=1))

    in_maps = []
    for c in range(N_CORES):
        in_maps.append({"xh": xh[c], "ds": ds})
    return in_maps


_MODULE_CACHE = {}


def kernel(**inputs):
    x = np.asarray(inputs["x"], dtype=np.float32)
    diagonals = np.asarray(inputs["diagonals"], dtype=np.float32)
    stride = int(np.asarray(inputs.get("stride", 1)))
    reps = int(np.asarray(inputs.get("reps", 1)))
    assert x.shape == (BATCH, SLOTS) and diagonals.shape == (NUM_T, SLOTS)
    # halo must cover the largest in-partition shift
    dec, _, _, _ = _decomp(stride, reps)
    assert max(df for _, df in dec) <= HALO, "halo too small for this stride"

    key = (stride, reps)
    if key not in _MODULE_CACHE:
        _MODULE_CACHE[key] = build_module(stride, reps)
    nc = _MODULE_CACHE[key]

    in_maps = prep_inputs(x, diagonals, stride, reps)
    res = run_bass_kernel_spmd(nc, in_maps, list(range(N_CORES)))
    out = np.concatenate(
        [np.asarray(res.results[c]["y"]) for c in range(N_CORES)], axis=0
    )
    return out.astype(np.float32)


# revision 20
# speedup vs baseline: 1.7182x; 1.0137x over previous
"""Trainium2 Bass kernel for the FHE BSGS conv-as-linear-transform problem.

Computes, for each batch row b of x[64, 65536]:
    out[b, s] = sum_{i=0}^{15} x[b, (s + 2^i * stride) % S] * diagonals[i, s]
    out = roll(out, S // (2 * reps))            (S = 65536)

Distribution: batch dim sharded across 8 NeuronCores (8 rows per core),
diagonals + stationary matrices replicated. No cross-core communication.

Per-core algorithm (free-minor layout: slot s = 512*p + f, p = partition):
  - A shift delta = 512*m + df decomposes into a free-dim column offset df
    (read the x tile at offset df against a 256-column halo) and a partition
    rotation by m (folded into the accumulation matmul).
  - All 16 elementwise products run on DVE in f16 (2x mode), fused over all
    8 batches via a broadcast diagonal operand and over transform groups in
    arithmetic progression (max 3 per op so the PE feed stays smooth and
    prod-pool buffers stay small).  Ops are split by batch half so compute
    starts after half the x tile has landed, and each half's last op is a
    singleton to keep the pipeline tail short.  (The Pool engine measures
    fine in isolation but destroys the pipeline on real HW — SBUF port
    contention with DVE — so everything stays on DVE.)
  - TensorE accumulates every product tile into a per-batch PSUM bank via a
    128x128 rotation-permutation stationary matmul; the partition rotation
    of each shift AND the final roll (multiple of 512 slots) are folded in,
    so PSUM holds the final rolled output directly.  PE matmuls are emitted
    in estimated product-availability order (engines run in-order), after a
    burst of warmup matmuls on a zero tile that ride out the PE p-state
    ramp during the DMA prologue.
  - PSUM eviction downcasts to f16 (ScalarE for the first half, ScalarE +
    DVE in parallel for the last banks); y is stored f16 and upcast on
    host.  Output DMA per batch half on otherwise-idle queues.

All device-input arrays are prepared host-side in the exact SBUF layout so
every input DMA is a dense partition-major copy.
"""

import os
import sys

for _p in ("/opt/trn_rl_repo",):
    if os.path.isdir(_p) and _p not in sys.path:
        sys.path.insert(0, _p)

import numpy as np

import concourse.bass as bass
import concourse.mybir as mybir
from concourse import bacc
from concourse.bass_utils import run_bass_kernel_spmd
from concourse.tile import TileContext

N_CORES = 8
BATCH = 64
SLOTS = 65536
NUM_T = 16
P = 128                 # SBUF partitions
F = SLOTS // P          # 512 slots per partition per batch
BPC = BATCH // N_CORES  # 8 batch rows per core
HALO = 256              # halo columns (covers df <= 256 for stride=1)
XPITCH = F + HALO       # 768
MAX_GROUP = int(os.environ.get("K_MAXG", "3"))           # max transforms fused into one DVE op
NS_COL_DVE = 0.521      # DVE f16 2x ns/col (cost model)
OP_OVH = 160.0          # per-op overhead estimate (ns)
N_WARMUP = 8            # PE warmup matmuls (ride out p-state ramp)

KNOB_POOL = int(os.environ.get("K_POOL", "0"))    # transforms on Pool engine
KNOB_BUFS = int(os.environ.get("K_BUFS", "6"))    # prod pool buffers


def _decomp(stride, reps):
    """Per-transform (m_i, df_i) shift decomposition + stationary rotations."""
    roll = (SLOTS // (2 * reps)) % SLOTS
    assert roll % F == 0, f"final roll {roll} not a multiple of {F}"
    mr = roll // F
    dec = []
    for i in range(NUM_T):
        delta = ((1 << i) * stride) % SLOTS
        dec.append((delta // F, delta % F))
    rots = [(m - mr) % P for (m, _) in dec]
    uniq = sorted(set(rots))
    sidx = {a: j for j, a in enumerate(uniq)}
    return dec, rots, uniq, sidx


def _plan(stride, reps):
    """Group transforms into DVE ops and fix the execution order.

    Returns (dec, rots, uniq, sidx, pool_idx, groups); each group is
    (base, step, members) — a fused DVE op reading x at offsets
    base + k*step (all even except a lone odd singleton is allowed).
    Execution order: a singleton first (fast PE start), then pairs and
    3-member groups interleaved, ending with the remaining singletons so
    each half's tail op is short.
    """
    dec, rots, uniq, sidx = _decomp(stride, reps)
    pool = []
    if KNOB_POOL > 0:
        zeros = [i for i in range(NUM_T) if dec[i][1] == 0]
        while len(pool) < KNOB_POOL and zeros:
            pool.append(zeros.pop())
    rest = [i for i in range(NUM_T) if i not in pool]
    rows = sorted(rest, key=lambda i: dec[i][1])
    groups = []
    k = 0
    while k < len(rows):
        base = dec[rows[k]][1]
        members = [rows[k]]
        step = None
        j = k + 1
        while j < len(rows) and len(members) < MAX_GROUP:
            d = dec[rows[j]][1] - dec[rows[j - 1]][1]
            if d % 2 != 0 or base % 2 != 0:
                break
            if step is None:
                step = d
            elif d != step:
                break
            members.append(rows[j])
            j += 1
        groups.append((base, step or 0, members))
        k = j
    singles = [g for g in groups if len(g[2]) == 1]
    multis = sorted([g for g in groups if len(g[2]) > 1],
                    key=lambda g: len(g[2]))
    order = []
    if singles:
        order.append(singles.pop(0))
    # interleave small multis with big ones
    lo = [g for g in multis if len(g[2]) <= 2]
    hi = [g for g in multis if len(g[2]) > 2]
    while lo or hi:
        if lo:
            order.append(lo.pop(0))
        if hi:
            order.append(hi.pop(0))
    order += singles
    return dec, rots, uniq, sidx, pool, order


def build_module(stride=1, reps=1, debug=False, repeat=1):
    """Build + finalize the per-core Bass module (same program on all cores)."""
    dec, rots, uniq, sidx, pool_idx, groups = _plan(stride, reps)
    ns = len(uniq)
    HB = BPC // 2  # batches per half

    # ds column layout: [first-2-ops diags | stats | rest in consumption
    # order].  chunk1 (DVE queue) feeds the first two ops + the PE
    # stationaries; chunk2/chunk3 (SP queue, after x half1) feed the rest.
    dd_order = [i for (_, _, mem) in groups for i in mem] + pool_idx
    n_front = sum(len(groups[k][2]) for k in range(min(2, len(groups))))
    dd_col = {i: k for k, i in enumerate(dd_order)}
    st0 = n_front * F
    dcol = lambda i: (dd_col[i] * F if dd_col[i] < n_front
                      else dd_col[i] * F + ns * P)
    ds_cols = NUM_T * F + ns * P
    n_c1 = st0 + ns * P
    n_c2 = min(n_c1 + 5 * F, ds_cols)

    f16 = mybir.dt.float16
    f32 = mybir.dt.float32

    nc = bacc.Bacc("TRN2", target_bir_lowering=False, debug=debug,
                   num_devices=N_CORES)
    xh = nc.dram_tensor("xh", [P, BPC * XPITCH], f16, kind="ExternalInput")
    ds = nc.dram_tensor("ds", [P, ds_cols], f16, kind="ExternalInput")
    y = nc.dram_tensor("y", [BPC, SLOTS], f16, kind="ExternalOutput")

    with TileContext(nc) as tc:
        with (
            tc.tile_pool(name="xa", bufs=1) as xa_pool,
            tc.tile_pool(name="dda", bufs=1) as dd_pool,
            tc.tile_pool(name="wu", bufs=1) as wu_pool,
            tc.tile_pool(name="prod", bufs=KNOB_BUFS) as prod_pool,
            tc.tile_pool(name="pprod", bufs=2) as pprod_pool,
            tc.tile_pool(name="outs", bufs=2) as out_pool,
            tc.tile_pool(name="ps", bufs=1, space="PSUM") as ps_pool,
        ):
            DS = dd_pool.tile([P, ds_cols], f16)
            X = xa_pool.tile([P, BPC * XPITCH], f16)
            WU = wu_pool.tile([P, F], f16)
            xq = 2 * XPITCH
            xmid = HB * XPITCH
            # Pool queue (SWDGE): chunk1 (first-2-ops diags + stats).  SP:
            # x batches 0-1 then 2-3 (the first product op covers batches
            # 0-1 only, so it starts after a quarter of x), then the
            # remaining diags.  Act: x half2 (behind the act-table load).
            nc.gpsimd.dma_start(out=DS[:, :n_c1], in_=ds[:, :n_c1])
            nc.sync.dma_start(out=X[:, :xq], in_=xh[:, :xq])
            nc.sync.dma_start(out=X[:, xq:xmid], in_=xh[:, xq:xmid])
            nc.scalar.dma_start(out=X[:, xmid:], in_=xh[:, xmid:])
            nc.sync.dma_start(out=DS[:, n_c1:n_c2], in_=ds[:, n_c1:n_c2])
            if n_c2 < ds_cols:
                nc.sync.dma_start(out=DS[:, n_c2:], in_=ds[:, n_c2:])
            nc.gpsimd.memset(WU[:], 0.0)

            max_prod = max(len(g[2]) for g in groups) * HB * F

            def body(_iv=None):
                psums = [
                    ps_pool.tile([P, F], f32, name=f"psum{b}", tag=f"ps{b}",
                                 bufs=1)
                    for b in range(BPC)
                ]
                # PE warmup: self-contained zero accumulation groups that
                # keep PE busy (and ramping) through the DMA prologue.
                for _ in range(N_WARMUP):
                    nc.tensor.matmul(psums[0][:], WU[:, :P], WU[:],
                                     start=True, stop=True)

                # op tuples: (b0, nb, base, step, mem).  Half split, except
                # the very first op (a singleton) runs as two batch-pair
                # quarters so compute starts after a quarter of x.
                dve_ops = []
                for h in (0, 1):
                    for gi, (base, step, mem) in enumerate(groups):
                        if h == 0 and gi == 0 and len(mem) == 1:
                            dve_ops.append((0, 2, base, step, mem))
                            dve_ops.append((2, 2, base, step, mem))
                        else:
                            dve_ops.append((h * HB, HB, base, step, mem))
                pool_ops = [(h * HB, HB, dec[i][1], 0, [i])
                            for h in (0, 1) for i in pool_idx]

                # estimated ready times (ns) for PE emission ordering
                ready = {}
                t = 2000.0
                for b0, nb, base, step, mem in dve_ops:
                    t += len(mem) * nb * F * NS_COL_DVE + OP_OVH
                    for i in mem:
                        ready[(i, b0)] = t
                t = 2500.0
                for b0, nb, base, step, mem in pool_ops:
                    t += len(mem) * nb * F * 1.99 + OP_OVH
                    ready[(mem[0], b0)] = t

                ptile = {}   # (i, b0) -> (tile, col0, nb)
                for ops, pool_, eng in ((dve_ops, prod_pool, nc.vector),
                                        (pool_ops, pprod_pool, nc.gpsimd)):
                    for b0, nb, base, step, mem in ops:
                        ng = len(mem)
                        prod = pool_.tile(
                            [P, ng * nb * F], f16, name="prod",
                            tag=f"prod{eng.engine.value}",
                            padded_shape=[P, max_prod])
                        in0 = bass.AP(
                            X.tensor,
                            X.offset + base + b0 * XPITCH,
                            [list(X.ap[0]), [step, ng], [XPITCH, nb], [1, F]],
                        )
                        c0 = dcol(mem[0])
                        in1 = bass.AP(
                            DS.tensor, DS.offset + c0,
                            [list(DS.ap[0]), [F, ng], [0, nb], [1, F]],
                        )
                        out4 = prod[:].rearrange("p (g b f) -> p g b f",
                                                 b=nb, f=F)
                        eng.tensor_mul(out4, in0, in1)
                        for k, i in enumerate(mem):
                            ptile[(i, b0)] = (prod, k * nb * F, nb)

                # PE matmuls in estimated-availability order
                order = sorted(ptile.keys(), key=lambda ib: ready[ib])
                seen = set()
                last = {}
                for b in range(BPC):
                    last[b] = [ib for ib in order
                               if ib[1] <= b < ib[1] + ptile[ib][2]][-1]
                for i, b0 in order:
                    prod, c0, nb = ptile[(i, b0)]
                    lhsT = DS[:, st0 + sidx[rots[i]] * P:
                              st0 + (sidx[rots[i]] + 1) * P]
                    for k in range(nb):
                        b = b0 + k
                        nc.tensor.matmul(
                            psums[b][:], lhsT,
                            prod[:, c0 + k * F:c0 + (k + 1) * F],
                            start=(b not in seen),
                            stop=((i, b0) == last[b]),
                        )
                        seen.add(b)

                # eviction + output DMA.  Half 1: all four banks on Act,
                # one y DMA on SP.  Half 2 (the tail): banks 4,5 on Act
                # and 6,7 on DVE in parallel, per-bank y DMAs on SP so
                # each bank ships as soon as its eviction lands.
                for h in range(2):
                    ot = out_pool.tile([P, HB * F], f16, name=f"ot{h}",
                                       tag=f"ot{h}")
                    for k in range(HB):
                        b = h * HB + k
                        dst = ot[:, k * F:(k + 1) * F]
                        if h == 1 and k >= 2:
                            nc.vector.tensor_copy(dst, psums[b][:])
                        else:
                            nc.scalar.copy(dst, psums[b][:])
                        if h == 1:
                            ydst = y[b:b + 1, :].rearrange(
                                "b (p f) -> p b f", f=F)
                            nc.sync.dma_start(
                                out=ydst,
                                in_=ot[:, k * F:(k + 1) * F].rearrange(
                                    "p (b f) -> p b f", f=F))
                    if h == 0:
                        ydst = y[:HB, :].rearrange("b (p f) -> p b f", f=F)
                        nc.sync.dma_start(out=ydst, in_=ot[:].rearrange(
                            "p (b f) -> p b f", f=F))

            if repeat == 1:
                body()
            else:
                with tc.For_i(0, repeat, 1):
                    body()
    nc.finalize()
    return nc


def prep_inputs(x, diagonals, stride=1, reps=1):
    """Host-side shard + relayout. Returns in_maps for run_bass_kernel_spmd."""
    dec, rots, uniq, sidx, pool_idx, groups = _plan(stride, reps)
    ns = len(uniq)

    x16 = np.ascontiguousarray(x, dtype=np.float16)
    # halo tiles in SBUF layout: xh[p, b*XPITCH + j] = x[b, (512p + j) % S]
    j = np.arange(XPITCH)
    idx = (np.arange(P)[:, None] * F + j[None, :]) % SLOTS
    xt = x16[:, idx]                       # [BATCH, P, XPITCH]
    xh = np.ascontiguousarray(
        np.stack([np.transpose(xt[c * BPC:(c + 1) * BPC], (1, 0, 2))
                  .reshape(P, BPC * XPITCH) for c in range(N_CORES)])
    )

    # ds: [first-2-ops diag blocks | stats | remaining diag blocks], with
    # each diag block pre-rotated along partitions by its shift's m.
    d16 = np.asarray(diagonals, dtype=np.float16).reshape(NUM_T, P, F)
    dd_order = [i for (_, _, mem) in groups for i in mem] + pool_idx
    n_front = sum(len(groups[k][2]) for k in range(min(2, len(groups))))
    ddl = [np.roll(d16[i], dec[i][0], axis=0) for i in dd_order]
    dd = np.transpose(np.stack(ddl), (1, 0, 2)).reshape(P, NUM_T * F)

    st = np.zeros((ns, P, P), np.float16)
    cols = np.arange(P)
    for k, a in enumerate(uniq):
        st[k, (cols + a) % P, cols] = 1.0
    st = np.transpose(st, (1, 0, 2)).reshape(P, ns * P)

    c1 = n_front * F
    ds = np.ascontiguousarray(
        np.concatenate([dd[:, :c1], st, dd[:, c1:]], axis=1))

    in_maps = []
    for c in range(N_CORES):
        in_maps.append({"xh": xh[c], "ds": ds})
    return in_maps


_MODULE_CACHE = {}


def kernel(**inputs):
    x = np.asarray(inputs["x"], dtype=np.float32)
    diagonals = np.asarray(inputs["diagonals"], dtype=np.float32)
    stride = int(np.asarray(inputs.get("stride", 1)))
    reps = int(np.asarray(inputs.get("reps", 1)))
    assert x.shape == (BATCH, SLOTS) and diagonals.shape == (NUM_T, SLOTS)
    # halo must cover the largest in-partition shift
    dec, _, _, _ = _decomp(stride, reps)
    assert max(df for _, df in dec) <= HALO, "halo too small for this stride"

    key = (stride, reps)
    if key not in _MODULE_CACHE:
        _MODULE_CACHE[key] = build_module(stride, reps)
    nc = _MODULE_CACHE[key]

    in_maps = prep_inputs(x, diagonals, stride, reps)
    res = run_bass_kernel_spmd(nc, in_maps, list(range(N_CORES)))
    out = np.concatenate(
        [np.asarray(res.results[c]["y"]) for c in range(N_CORES)], axis=0
    )
    return out.astype(np.float32)
